# revision 3
# baseline (speedup 1.0000x reference)
"""LightGCN message-passing kernel for 8 TRN2 NeuronCores — v2.

v2 adds backward pruning: the final output only needs light_out at the
4096 batch slots (users + items), so
  - L1 (emb1 = A@emb0) runs over all 2M edges (emb1 needed ~everywhere),
  - L2 (emb2 = A@emb1) only at S2 = batch nodes + their in-neighbors
    (~54K nodes, ~720K edges),
  - L3 (emb3 = A@emb2) only at the 4096 batch slots (~54K edges),
and acc = emb0+emb1+emb2 is read at batch slots via synthetic val=1
"edges" into the slot segment-sum. This cuts per-edge dma_gather
descriptor generation (the GPSIMD bottleneck) by ~2.1x.

Mechanics per layer are the baseline's: dest-row sharding, dma_gather of
source rows, VectorE scale + one-hot build, TensorE segment-sum into
PSUM (one accumulation group per PSUM bank), AllGather between layers.
Pad slots carry val=0 so garbage gathers are harmless.
"""

import sys

sys.path.insert(0, "/opt/trn_rl_repo")

import numpy as np
import ml_dtypes

import concourse.bacc as bacc
import concourse.bass as bass
import concourse.mybir as mybir
import concourse.tile as tile
from concourse.bass_utils import run_bass_kernel_spmd
from concourse import library_config

# ---------------------------------------------------------------- constants
N_USER = 100000
N_ITEM = 50000
N_NODES = 150000
N_EDGES = 2000000
EMB = 64
N_LAYERS = 3
BATCH = 2048
NC = 8

SHARD = N_NODES // NC            # 18750 logical rows per core
BLK = 128                        # dest rows per block (PSUM partitions)
NBLK = (SHARD + BLK - 1) // BLK  # 147 blocks per core
SHARD_PAD = NBLK * BLK           # 18816 physical rows per core
NPHYS = NC * SHARD_PAD           # 150528
NBANK = 5
BANKROWS = (NPHYS + NBANK - 1) // NBANK  # 30106 (< 32768 for int16)
SBLK = 16                        # blocks per superblock
NSB = (NBLK + SBLK - 1) // SBLK  # 10

OUT_ROWS = BATCH // NC           # 256 output user-rows per core
GSUB = 8                         # chunks per sub-gather (1024 indices)

F32 = mybir.dt.float32
BF16 = mybir.dt.bfloat16
I16 = mybir.dt.int16

_BF16NP = ml_dtypes.bfloat16


def _phys(node):
    """Global node id -> physical table row (per-core (p, blk) layout)."""
    node = np.asarray(node, dtype=np.int64)
    m = node // SHARD
    r = node - m * SHARD
    blk = r // BLK
    p = r - blk * BLK
    return m * SHARD_PAD + p * NBLK + blk


def _wrap_idx(seq):
    """Flat int16 index sequence -> [128, len/16] wrapped+replicated layout."""
    n = len(seq)
    assert n % 16 == 0
    w = np.asarray(seq, dtype=np.int16).reshape(-1, 16).T  # [16, n/16]
    return np.tile(w, (8, 1)).astype(np.int16)


def _prep_pass(dest_local, src_loc, src_bank, vals, dest_core, nblk, nbank, sblk):
    """Build per-core uniform chunk structure for one segment-sum pass.

    dest_local: local dest row within the owning core's range [0, nblk*128)
    src_loc:    source row within its bank (int, < 32768)
    src_bank:   source bank id
    vals:       edge values (float32)
    dest_core:  owning core of each edge
    Returns (meta, per-core arrays (idx16, valf, rbyte16)) where meta has
    gather groups [(slot_off, n_idx, bank)] and per-gather chunk lists
    [(blk, seg_jj)].
    """
    nseg = nblk * nbank
    counts = np.zeros((NC, nseg), dtype=np.int64)
    per_core = []
    for m in range(NC):
        sel = dest_core == m
        dl = dest_local[sel]
        blk = dl // BLK
        p = dl - blk * BLK
        key = blk * nbank + src_bank[sel]
        order = np.argsort(key, kind="stable")
        per_core.append(
            dict(key=key[order], loc=src_loc[sel][order], p=p[order], val=vals[sel][order])
        )
        counts[m] = np.bincount(key, minlength=nseg)

    cmax = counts.max(axis=0)
    C_seg = (cmax + BLK - 1) // BLK  # chunks per segment (uniform across cores)

    nsb = (nblk + sblk - 1) // sblk
    maxC = int(C_seg.max()) if len(C_seg) else 0
    # gather groups: one per (sb, bank); chunks within a group ordered by
    # (jj, blk) so emptier chunks sink to the gather tail (trailing -1 strip)
    gathers = []       # (slot_off, n_slots, bank)
    chunk_meta = []    # per gather: [(blk, jj), ...]
    # chunk_base_arr[seg, jj] -> slot offset of chunk jj of segment seg
    chunk_base_arr = np.full((nseg, max(maxC, 1)), -1, dtype=np.int64)
    pos = 0
    for sb in range(nsb):
        blks = list(range(sb * sblk, min((sb + 1) * sblk, nblk)))
        for b in range(nbank):
            meta = []
            maxjj = max((int(C_seg[blk * nbank + b]) for blk in blks), default=0)
            start = pos
            for jj in range(maxjj):
                for blk in blks:
                    if jj < C_seg[blk * nbank + b]:
                        meta.append((blk, jj))
                        chunk_base_arr[blk * nbank + b, jj] = pos
                        pos += BLK
            gathers.append((start, pos - start, b))
            chunk_meta.append(meta)
    S_total = pos

    core_arrays = []
    for m in range(NC):
        d = per_core[m]
        key = d["key"]
        nedge = len(key)
        first_of_key = np.zeros(nseg, dtype=np.int64)
        cnts = np.bincount(key, minlength=nseg)
        first_of_key[1:] = np.cumsum(cnts)[:-1]
        rank = np.arange(nedge) - first_of_key[key]
        jj = rank // BLK
        slot = chunk_base_arr[key, jj] + (rank - jj * BLK)
        assert (slot >= 0).all()

        idx16 = np.zeros(S_total, dtype=np.int16)
        valf = np.zeros(S_total, dtype=np.float32)
        rbyte = np.zeros(S_total, dtype=np.int16)
        idx16[slot] = d["loc"].astype(np.int16)
        valf[slot] = d["val"]
        rbyte[slot] = d["p"].astype(np.int16)
        core_arrays.append((idx16, valf, rbyte))

    meta = dict(
        gathers=gathers, chunk_meta=chunk_meta, S_total=S_total,
        nblk=nblk, nbank=nbank, sblk=sblk,
    )
    return meta, core_arrays


def _slot_cols(slot_arrays, S_total):
    """Stack per-core slot arrays into device input layouts."""
    outs = []
    for (idx16, valf, rbyte) in slot_arrays:
        idx_w = _wrap_idx(idx16)
        vals_t = valf.reshape(-1, BLK).T.copy()
        rbyte_t = rbyte.reshape(-1, BLK).T.astype(_BF16NP)
        outs.append((idx_w, vals_t, rbyte_t))
    return outs


def _prep_graph(adj_vals, adj_rows, adj_cols, users, items):
    rows = np.asarray(adj_rows, dtype=np.int64)
    cols = np.asarray(adj_cols, dtype=np.int64)
    vals = np.asarray(adj_vals, dtype=np.float32)
    users = np.asarray(users, dtype=np.int64)
    items = np.asarray(items, dtype=np.int64)

    t_nodes = np.concatenate([users, N_USER + items])  # [4096] with dups
    NSLOT = len(t_nodes)

    # ---- L3-proper edges: edges into t_nodes, replicated per slot
    t_order = np.argsort(t_nodes, kind="stable")
    t_sorted = t_nodes[t_order]
    lo = np.searchsorted(t_sorted, rows, side="left")
    hi = np.searchsorted(t_sorted, rows, side="right")
    nrep = hi - lo                       # slots per edge (mostly 0)
    esel = np.nonzero(nrep)[0]
    rep = nrep[esel]
    e_idx = np.repeat(esel, rep)         # edge index per L3 edge-instance
    # slot (sorted order) per instance
    starts = lo[esel]
    offs = np.arange(len(e_idx)) - np.repeat(
        np.concatenate([[0], np.cumsum(rep)[:-1]]), rep
    )
    slot3 = t_order[starts.repeat(rep) + offs]
    src3 = cols[e_idx]
    val3 = vals[e_idx]

    # ---- S2 = batch nodes + sources of L3 edges
    s2_mask = np.zeros(N_NODES, dtype=bool)
    s2_mask[t_nodes] = True
    s2_mask[src3] = True

    # ---- L2 edges: dest in S2
    sel2 = s2_mask[rows]
    rows2, cols2, vals2 = rows[sel2], cols[sel2], vals[sel2]

    # ---- slot sharding: slot -> home core of its node; local slot index
    slot_core = t_nodes // SHARD
    slot_local = np.zeros(NSLOT, dtype=np.int64)
    core_slot_count = np.zeros(NC, dtype=np.int64)
    for m in range(NC):
        sl = np.nonzero(slot_core == m)[0]
        slot_local[sl] = np.arange(len(sl))
        core_slot_count[m] = len(sl)
    SLOTP = int(-(-core_slot_count.max() // BLK) * BLK)
    NBLK3 = SLOTP // BLK

    # ---- pass structures
    sphys = _phys(cols)
    bank_all = np.minimum(sphys // BANKROWS, NBANK - 1)
    loc_all = sphys - bank_all * BANKROWS

    metaL1, arrL1 = _prep_pass(
        rows - (rows // SHARD) * SHARD,
        loc_all, bank_all, vals, rows // SHARD, NBLK, NBANK, SBLK,
    )
    sphys2 = _phys(cols2)
    bank2 = np.minimum(sphys2 // BANKROWS, NBANK - 1)
    loc2 = sphys2 - bank2 * BANKROWS
    metaL2, arrL2 = _prep_pass(
        rows2 - (rows2 // SHARD) * SHARD,
        loc2, bank2, vals2, rows2 // SHARD, NBLK, NBANK, SBLK,
    )
    # L3 proper: dest = local slot, src from full table2 (5 banks)
    sphys3 = _phys(src3)
    bank3 = np.minimum(sphys3 // BANKROWS, NBANK - 1)
    loc3 = sphys3 - bank3 * BANKROWS
    metaL3, arrL3 = _prep_pass(
        slot_local[slot3], loc3, bank3, val3, slot_core[slot3], NBLK3, NBANK, NBLK3,
    )
    # L3 synthetic: dest = local slot, src = LOCAL acc shard row, val = 1
    syn_slot = np.arange(NSLOT)
    syn_src_phys = _phys(t_nodes)                  # global phys row
    syn_loc = syn_src_phys - (syn_src_phys // SHARD_PAD) * SHARD_PAD  # local row
    metaSyn, arrSyn = _prep_pass(
        slot_local[syn_slot],
        syn_loc, np.zeros(NSLOT, dtype=np.int64), np.ones(NSLOT, dtype=np.float32),
        slot_core[syn_slot], NBLK3, 1, NBLK3,
    )

    # ---- final extraction indices
    def bounce_row(l):
        return (l % BLK) * NBLK3 + l // BLK

    exrow = slot_core * SLOTP + bounce_row(slot_local)  # ex_full row per slot
    exu = np.zeros((NC, OUT_ROWS), dtype=np.int16)
    for m in range(NC):
        exu[m] = exrow[m * OUT_ROWS : (m + 1) * OUT_ROWS].astype(np.int16)
    exi = exrow[BATCH:].astype(np.int16)  # item slots, same for all cores

    return dict(
        metaL1=metaL1, arrL1=arrL1,
        metaL2=metaL2, arrL2=arrL2,
        metaL3=metaL3, arrL3=arrL3,
        metaSyn=metaSyn, arrSyn=arrSyn,
        SLOTP=SLOTP, NBLK3=NBLK3, exu=exu, exi=exi,
    )


def _build(g):
    """Build the SPMD Bass graph (identical for all cores)."""
    metaL1, metaL2 = g["metaL1"], g["metaL2"]
    metaL3, metaSyn = g["metaL3"], g["metaSyn"]
    SLOTP, NBLK3 = g["SLOTP"], g["NBLK3"]
    S1, S2_, S3, SS = (
        metaL1["S_total"], metaL2["S_total"], metaL3["S_total"], metaSyn["S_total"],
    )

    nc = bacc.Bacc("TRN2", target_bir_lowering=False)

    table0 = nc.declare_dram_parameter("table0", [NPHYS, EMB], F32, isOutput=False)
    acc0 = nc.declare_dram_parameter("acc0", [SHARD_PAD, EMB], F32, isOutput=False)
    idx1_in = nc.declare_dram_parameter("idx1", [128, S1 // 16], I16, isOutput=False)
    vals1_in = nc.declare_dram_parameter("vals1", [128, S1 // 128], F32, isOutput=False)
    rb1_in = nc.declare_dram_parameter("rb1", [128, S1 // 128], BF16, isOutput=False)
    idx2_in = nc.declare_dram_parameter("idx2", [128, S2_ // 16], I16, isOutput=False)
    vals2_in = nc.declare_dram_parameter("vals2", [128, S2_ // 128], F32, isOutput=False)
    rb2_in = nc.declare_dram_parameter("rb2", [128, S2_ // 128], BF16, isOutput=False)
    idx3_in = nc.declare_dram_parameter("idx3", [128, S3 // 16], I16, isOutput=False)
    vals3_in = nc.declare_dram_parameter("vals3", [128, S3 // 128], F32, isOutput=False)
    rb3_in = nc.declare_dram_parameter("rb3", [128, S3 // 128], BF16, isOutput=False)
    idxs_in = nc.declare_dram_parameter("idxs", [128, SS // 16], I16, isOutput=False)
    valss_in = nc.declare_dram_parameter("valss", [128, SS // 128], F32, isOutput=False)
    rbs_in = nc.declare_dram_parameter("rbs", [128, SS // 128], BF16, isOutput=False)
    iota_in = nc.declare_dram_parameter("iota", [128, 128], BF16, isOutput=False)
    ident_in = nc.declare_dram_parameter("ident", [128, 128], F32, isOutput=False)
    exu_in = nc.declare_dram_parameter("exu", [128, OUT_ROWS // 16], I16, isOutput=False)
    exi_in = nc.declare_dram_parameter("exi", [128, BATCH // 16], I16, isOutput=False)
    out_ext = nc.declare_dram_parameter("out", [OUT_ROWS, BATCH], F32, isOutput=True)

    with tile.TileContext(nc) as tc:
        nc.gpsimd.load_library(library_config.mlp)
        with (
            tc.tile_pool(name="const", bufs=1) as constp,
            tc.tile_pool(name="dram", bufs=1, space="DRAM") as dramp,
            tc.tile_pool(name="gpool", bufs=6) as gpool,
            tc.tile_pool(name="gspool", bufs=3) as gspool,
            tc.tile_pool(name="p01pool", bufs=3) as p01pool,
            tc.tile_pool(name="psum", bufs=2, space="PSUM") as psump,
            tc.tile_pool(name="fin", bufs=2) as finp,
            tc.tile_pool(name="fpsum", bufs=2, space="PSUM") as fpsump,
        ):
            tables = [table0]
            shard_bounces = []
            for l in range(2):
                sb_t = dramp.tile([SHARD_PAD, EMB], F32, name=f"shardb{l}")
                shard_bounces.append(sb_t)
                tb_t = dramp.tile([NPHYS, EMB], F32, addr_space="Shared", name=f"tableb{l + 1}")
                tables.append(tb_t)
            acc_dram = dramp.tile([SHARD_PAD, EMB], F32, name="acc_dram")
            ex_bounce = dramp.tile([SLOTP, EMB], F32, name="ex_bounce")
            ex_full = dramp.tile([NC * SLOTP, EMB], F32, addr_space="Shared", name="ex_full")

            idx1_sb = constp.tile([128, S1 // 16], I16)
            vals1_sb = constp.tile([128, S1 // 128], F32)
            rb1_sb = constp.tile([128, S1 // 128], BF16)
            idx2_sb = constp.tile([128, S2_ // 16], I16)
            vals2_sb = constp.tile([128, S2_ // 128], F32)
            rb2_sb = constp.tile([128, S2_ // 128], BF16)
            idx3_sb = constp.tile([128, S3 // 16], I16)
            vals3_sb = constp.tile([128, S3 // 128], F32)
            rb3_sb = constp.tile([128, S3 // 128], BF16)
            idxs_sb = constp.tile([128, SS // 16], I16)
            valss_sb = constp.tile([128, SS // 128], F32)
            rbs_sb = constp.tile([128, SS // 128], BF16)
            iota_sb = constp.tile([128, 128], BF16)
            ident_sb = constp.tile([128, 128], F32)
            acc_sb = constp.tile([128, NBLK * EMB], F32)
            dummy16 = constp.tile([128, 1], I16)
            dummyf = constp.tile([128, 1], F32)

            for sb_, in_ in (
                (idx1_sb, idx1_in), (vals1_sb, vals1_in), (rb1_sb, rb1_in),
                (idx2_sb, idx2_in), (vals2_sb, vals2_in), (rb2_sb, rb2_in),
                (idx3_sb, idx3_in), (vals3_sb, vals3_in), (rb3_sb, rb3_in),
                (idxs_sb, idxs_in), (valss_sb, valss_in), (rbs_sb, rbs_in),
                (iota_sb, iota_in), (ident_sb, ident_in),
            ):
                nc.sync.dma_start(sb_[:], in_[:])
            nc.sync.dma_start(
                acc_sb[:], acc0[:, :].rearrange("(p x) e -> p (x e)", p=128)
            )

            ghist = []

            def issue_gather(g_tile, col_off, nch_sub, src_ap, idx_slice):
                # reclaim window: wait the gather 5 back (gpool bufs=6, so
                # buffer reuse at i-6 is still covered) — deeper than the
                # minimum so issue doesn't stall on in-flight DMA latency
                if len(ghist) >= 5:
                    pt, po = ghist[-5]
                    nc.gpsimd.tensor_copy(out=dummyf[:, :1], in_=pt[:, po : po + 1])
                nc.gpsimd.memset(g_tile[:, col_off : col_off + 1], 0.0)
                nc.gpsimd.dma_gather(
                    out_ap=g_tile[
                        :, col_off : col_off + nch_sub * EMB
                    ].rearrange("p (c e) -> p c e", e=EMB),
                    in_ap=src_ap,
                    idxs_ap=idx_slice,
                    num_idxs=nch_sub * BLK,
                    num_idxs_reg=nch_sub * BLK,
                    elem_size=EMB,
                )
                ghist.append((g_tile, col_off))

            # absorbers for idx staging dependencies
            for t in (idx1_sb, idx2_sb, idx3_sb, idxs_sb):
                nc.gpsimd.tensor_copy(out=dummy16[:, :1], in_=t[:, :1])

            # pre-zero the gather buffers: trailing-stripped (-1) slots are
            # never written by the DMA, and 0 * garbage could be NaN
            for w in range(6):
                wt = gpool.tile([128, GSUB * EMB], F32, tag="g", name=f"gwarm_{w}")
                nc.vector.memset(wt[:], 0.0)

            def run_pass(meta, idx_sb, vals_sb, rb_sb, src_of_bank, psum_of_blk,
                         flags, lname):
                """Emit gathers + scale + one-hot + segment-sum matmuls.

                psum_of_blk(blk) -> (psum_tile, col); flags[(blk)] counts
                handled externally via `flags` dict {blk: [seen, total]}.
                """
                gathers, chunk_meta = meta["gathers"], meta["chunk_meta"]
                for gi, (off, n_idx, bank) in enumerate(gathers):
                    if n_idx == 0:
                        continue
                    nch = n_idx // BLK
                    src_ap = src_of_bank(bank)
                    nsub = (nch + GSUB - 1) // GSUB
                    for sg in range(nsub):
                        c_lo = sg * GSUB
                        nch_sub = min(GSUB, nch - c_lo)
                        goff = off + c_lo * BLK
                        gt = gpool.tile([128, GSUB * EMB], F32, tag="g", name=f"g_{lname}_{gi}_{sg}")
                        issue_gather(
                            gt, 0, nch_sub, src_ap,
                            idx_sb[:, goff // 16 : (goff + nch_sub * BLK) // 16],
                        )
                        gs = gspool.tile([128, GSUB * EMB], BF16, tag="gs", name=f"gs_{lname}_{gi}_{sg}")
                        c0 = goff // BLK
                        nc.vector.tensor_tensor(
                            out=gs[:, : nch_sub * EMB].rearrange("p (c e) -> p c e", e=EMB),
                            in0=gt[:, : nch_sub * EMB].rearrange("p (c e) -> p c e", e=EMB),
                            in1=vals_sb[:, c0 : c0 + nch_sub]
                            .rearrange("p (c o) -> p c o", o=1)
                            .to_broadcast([128, nch_sub, EMB]),
                            op=mybir.AluOpType.mult,
                        )
                        p01 = p01pool.tile([128, GSUB * 128], BF16, tag="p01", name=f"p01_{lname}_{gi}_{sg}")
                        nc.vector.tensor_tensor(
                            out=p01[:, : nch_sub * 128].rearrange("p (c q) -> p c q", q=128),
                            in0=rb_sb[:, c0 : c0 + nch_sub]
                            .rearrange("p (c o) -> p c o", o=1)
                            .to_broadcast([128, nch_sub, 128]),
                            in1=iota_sb[:, :]
                            .rearrange("p (o q) -> p o q", o=1)
                            .to_broadcast([128, nch_sub, 128]),
                            op=mybir.AluOpType.is_equal,
                        )
                        for jj_local in range(nch_sub):
                            j = c_lo + jj_local
                            blk, _jj = chunk_meta[gi][j]
                            ph, col, hkey = psum_of_blk(blk)
                            seen, total = flags[hkey]
                            nc.tensor.matmul(
                                out=ph[:, col * EMB : (col + 1) * EMB],
                                lhsT=p01[:, jj_local * 128 : (jj_local + 1) * 128],
                                rhs=gs[:, jj_local * EMB : (jj_local + 1) * EMB],
                                start=(seen == 0),
                                stop=(seen == total - 1),
                                skip_group_check=True,
                            )
                            flags[hkey][0] += 1

            # ================= L1 and L2 =================
            for l, (meta, isb, vsb, rsb) in enumerate((
                (metaL1, idx1_sb, vals1_sb, rb1_sb),
                (metaL2, idx2_sb, vals2_sb, rb2_sb),
            )):
                src = tables[l]
                gathers, chunk_meta = meta["gathers"], meta["chunk_meta"]
                ngather_per_sb = NBANK  # groups per superblock
                for sb in range(NSB):
                    blks = list(range(sb * SBLK, min((sb + 1) * SBLK, NBLK)))
                    nhalf = (len(blks) + 7) // 8
                    halves = [
                        psump.tile(
                            [128, min(8, len(blks) - 8 * h) * EMB], F32,
                            tag=f"ph{h}", name=f"ph_{l}_{sb}_{h}",
                        )
                        for h in range(nhalf)
                    ]
                    flags = {}
                    gsl = list(range(sb * ngather_per_sb, (sb + 1) * ngather_per_sb))
                    for gi in gsl:
                        for (blk, _jj) in chunk_meta[gi]:
                            h = (blk - sb * SBLK) // 8
                            flags.setdefault(h, [0, 0])[1] += 1

                    def psum_of_blk(blk, sb=sb, halves=halves):
                        h = (blk - sb * SBLK) // 8
                        return halves[h], (blk - sb * SBLK) % 8, h

                    sub_meta = dict(
                        gathers=[gathers[gi] for gi in gsl],
                        chunk_meta=[chunk_meta[gi] for gi in gsl],
                    )
                    run_pass(
                        sub_meta, isb, vsb, rsb,
                        lambda bank, src=src: src[
                            bank * BANKROWS : bank * BANKROWS + min(BANKROWS, NPHYS - bank * BANKROWS), :
                        ],
                        psum_of_blk, flags, f"l{l}s{sb}",
                    )
                    # drain superblock PSUM
                    for h, ph in enumerate(halves):
                        b0 = sb * SBLK + h * 8
                        nb = ph.shape[1] // EMB
                        if flags.get(h, [0, 0])[1] > 0:
                            nc.vector.tensor_tensor(
                                out=acc_sb[:, b0 * EMB : (b0 + nb) * EMB],
                                in0=acc_sb[:, b0 * EMB : (b0 + nb) * EMB],
                                in1=ph[:, :],
                                op=mybir.AluOpType.add,
                            )
                            lay = finp.tile([128, 8 * EMB], F32, tag="lay", name=f"lay_{l}_{sb}_{h}")
                            nc.scalar.copy(out=lay[:, : nb * EMB], in_=ph[:, :])
                            nc.sync.dma_start(
                                shard_bounces[l][:, :]
                                .rearrange("(p x) e -> p x e", p=128)[:, b0 : b0 + nb, :],
                                lay[:, : nb * EMB].rearrange("p (x e) -> p x e", e=EMB),
                            )
                nc.gpsimd.collective_compute(
                    "AllGather",
                    mybir.AluOpType.bypass,
                    ins=[shard_bounces[l][:, :].opt()],
                    outs=[tables[l + 1][:, :].opt()],
                    replica_groups=[list(range(NC))],
                )

            # write acc (= emb0+emb1+emb2 at this core's shard) for synthetic reads
            nc.sync.dma_start(
                acc_dram[:, :].rearrange("(p x) e -> p (x e)", p=128), acc_sb[:]
            )

            # ================= L3 slots (proper + synthetic) =================
            slot_psum = psump.tile([128, NBLK3 * EMB], F32, tag="ph0", name="slotp")
            nchunks3 = sum(len(c) for c in metaL3["chunk_meta"]) + sum(
                len(c) for c in metaSyn["chunk_meta"]
            )
            flags3 = {0: [0, nchunks3]}

            def psum_of_slot_blk(blk):
                return slot_psum, blk, 0

            run_pass(
                metaL3, idx3_sb, vals3_sb, rb3_sb,
                lambda bank: tables[2][
                    bank * BANKROWS : bank * BANKROWS + min(BANKROWS, NPHYS - bank * BANKROWS), :
                ],
                psum_of_slot_blk, flags3, "l3",
            )
            run_pass(
                metaSyn, idxs_sb, valss_sb, rbs_sb,
                lambda bank: acc_dram[:, :],
                psum_of_slot_blk, flags3, "syn",
            )

            # drain slot PSUM -> ex_bounce -> AllGather
            slot_sb = finp.tile([128, NBLK3 * EMB], F32, tag="slot_sb")
            nc.scalar.copy(out=slot_sb[:, :], in_=slot_psum[:, :])
            nc.sync.dma_start(
                ex_bounce[:, :].rearrange("(p x) e -> p (x e)", p=128), slot_sb[:]
            )
            nc.gpsimd.collective_compute(
                "AllGather",
                mybir.AluOpType.bypass,
                ins=[ex_bounce[:, :].opt()],
                outs=[ex_full[:, :].opt()],
                replica_groups=[list(range(NC))],
            )

            # ================= final extraction + GEMM =================
            exu_sb = finp.tile([128, OUT_ROWS // 16], I16, tag="exu")
            exi_sb = finp.tile([128, BATCH // 16], I16, tag="exi")
            nc.sync.dma_start(exu_sb[:], exu_in[:])
            nc.sync.dma_start(exi_sb[:], exi_in[:])
            nc.gpsimd.tensor_copy(out=dummy16[:, :1], in_=exu_sb[:, :1])
            nc.gpsimd.tensor_copy(out=dummy16[:, :1], in_=exi_sb[:, :1])

            u_sb = finp.tile([128, (OUT_ROWS // 128) * EMB], F32, tag="u")
            i_sb = finp.tile([128, (BATCH // 128) * EMB], F32, tag="i")
            issue_gather(u_sb, 0, OUT_ROWS // BLK, ex_full[:, :], exu_sb[:, :])
            for part in range(2):
                issue_gather(
                    i_sb, part * 8 * EMB, 8, ex_full[:, :],
                    exi_sb[:, part * 64 : (part + 1) * 64],
                )
            ut = finp.tile([64, (OUT_ROWS // 128) * 128], BF16, tag="ut")
            it = finp.tile([64, (BATCH // 128) * 128], BF16, tag="it")
            for t in range(OUT_ROWS // 128):
                tp = fpsump.tile([64, 128], F32, tag="tp", name=f"tpu_{t}")
                nc.tensor.transpose(out=tp[:, :], in_=u_sb[:, t * EMB : (t + 1) * EMB], identity=ident_sb[:, :])
                nc.vector.tensor_copy(out=ut[:, t * 128 : (t + 1) * 128], in_=tp[:, :])
            for t in range(BATCH // 128):
                tp = fpsump.tile([64, 128], F32, tag="tp", name=f"tpi_{t}")
                nc.tensor.transpose(out=tp[:, :], in_=i_sb[:, t * EMB : (t + 1) * EMB], identity=ident_sb[:, :])
                nc.vector.tensor_copy(out=it[:, t * 128 : (t + 1) * 128], in_=tp[:, :])
            for t in range(OUT_ROWS // 128):
                for q in range(BATCH // 512):
                    po = fpsump.tile([128, 512], F32, tag="po", name=f"po_{t}_{q}")
                    nc.tensor.matmul(
                        out=po[:, :],
                        lhsT=ut[:, t * 128 : (t + 1) * 128],
                        rhs=it[:, q * 512 : (q + 1) * 512],
                        start=True, stop=True,
                    )
                    ob = finp.tile([128, 512], F32, tag="ob", name=f"ob_{t}_{q}")
                    nc.scalar.activation(
                        out=ob[:, :], in_=po[:, :],
                        func=mybir.ActivationFunctionType.Sigmoid,
                        scale=1.0 / ((N_LAYERS + 1) ** 2),
                    )
                    nc.sync.dma_start(
                        out_ext[t * 128 : (t + 1) * 128, q * 512 : (q + 1) * 512],
                        ob[:, :],
                    )
    nc.compile()
    return nc


LAST_EXEC_NS = None
LAST_RES = None


def _ensure_trace_hook():
    """Install the axon NTFF profile hook if the image's antenv lacks it.

    Mirrors trn_agent_boot.trn_boot's step 6 (which degrades silently when
    antenv.axon_hooks is missing). Best-effort: any failure leaves tracing
    disabled, which run_bass_kernel_spmd already tolerates.
    """
    try:
        from antenv.axon_hooks import get_axon_ntff_profile_hook  # noqa: F401

        return  # real module present; boot already handled it
    except ImportError:
        pass
    try:
        import contextlib
        import ctypes
        import types

        import antenv

        lib = ctypes.CDLL("/opt/axon/libaxon_pjrt.so")
        if not hasattr(lib, "axon_start_nrt_profile"):
            return
        lib.axon_start_nrt_profile.argtypes = [
            ctypes.POINTER(ctypes.c_int64),
            ctypes.c_size_t,
        ]
        lib.axon_start_nrt_profile.restype = ctypes.c_int64
        lib.axon_stop_nrt_profile.argtypes = [ctypes.c_char_p]
        lib.axon_stop_nrt_profile.restype = ctypes.c_int64

        @contextlib.contextmanager
        def _hook(output_dir, device_ids):
            import jax

            jax.devices()
            if device_ids:
                ids = (ctypes.c_int64 * len(device_ids))(*device_ids)
                rc = lib.axon_start_nrt_profile(ids, len(device_ids))
            else:
                rc = lib.axon_start_nrt_profile(None, 0)
            if rc != 0:
                raise RuntimeError(f"axon_start_nrt_profile rc={rc}")
            try:
                yield
            finally:
                n = lib.axon_stop_nrt_profile(str(output_dir).encode())
                if n <= 0:
                    print(f"profile: {n} ntff files in {output_dir}")

        mod = types.ModuleType("antenv.axon_hooks")
        mod._hook = _hook
        mod.get_axon_ntff_profile_hook = lambda: mod._hook
        mod.set_axon_ntff_profile_hook = lambda h: setattr(mod, "_hook", h)
        sys.modules["antenv.axon_hooks"] = mod
        antenv.axon_hooks = mod
    except Exception:
        pass


def kernel(user_emb, item_emb, adj_vals, adj_rows, adj_cols, users, items):
    global LAST_EXEC_NS, LAST_RES
    user_emb = np.asarray(user_emb, dtype=np.float32)
    item_emb = np.asarray(item_emb, dtype=np.float32)

    g = _prep_graph(adj_vals, adj_rows, adj_cols, users, items)

    all_emb = np.concatenate([user_emb, item_emb], axis=0)
    table0 = np.zeros((NPHYS, EMB), dtype=np.float32)
    table0[_phys(np.arange(N_NODES))] = all_emb

    iota = np.tile(np.arange(128, dtype=_BF16NP)[None, :], (128, 1))
    ident = np.eye(128, dtype=np.float32)

    nc = _build(g)

    colsL1 = _slot_cols(g["arrL1"], g["metaL1"]["S_total"])
    colsL2 = _slot_cols(g["arrL2"], g["metaL2"]["S_total"])
    colsL3 = _slot_cols(g["arrL3"], g["metaL3"]["S_total"])
    colsSyn = _slot_cols(g["arrSyn"], g["metaSyn"]["S_total"])

    in_maps = []
    for m in range(NC):
        i1, v1, r1 = colsL1[m]
        i2, v2, r2 = colsL2[m]
        i3, v3, r3 = colsL3[m]
        isn, vsn, rsn = colsSyn[m]
        in_maps.append(
            {
                "table0": table0,
                "acc0": table0[m * SHARD_PAD : (m + 1) * SHARD_PAD],
                "idx1": i1, "vals1": v1, "rb1": r1,
                "idx2": i2, "vals2": v2, "rb2": r2,
                "idx3": i3, "vals3": v3, "rb3": r3,
                "idxs": isn, "valss": vsn, "rbs": rsn,
                "iota": iota, "ident": ident,
                "exu": _wrap_idx(g["exu"][m]),
                "exi": _wrap_idx(g["exi"]),
            }
        )

    _ensure_trace_hook()
    try:
        res = run_bass_kernel_spmd(nc, in_maps, core_ids=list(range(NC)), trace=True)
        LAST_EXEC_NS = res.exec_time_ns
    except Exception:
        res = run_bass_kernel_spmd(nc, in_maps, core_ids=list(range(NC)))
        LAST_EXEC_NS = None
    LAST_RES = res
    out = np.concatenate([res.results[m]["out"] for m in range(NC)], axis=0)
    return out.astype(np.float32)



# revision 7
# speedup vs baseline: 1.0511x; 1.0511x over previous
"""LightGCN message-passing kernel for 8 TRN2 NeuronCores — v2.

v2 adds backward pruning: the final output only needs light_out at the
4096 batch slots (users + items), so
  - L1 (emb1 = A@emb0) runs over all 2M edges (emb1 needed ~everywhere),
  - L2 (emb2 = A@emb1) only at S2 = batch nodes + their in-neighbors
    (~54K nodes, ~720K edges),
  - L3 (emb3 = A@emb2) only at the 4096 batch slots (~54K edges),
and acc = emb0+emb1+emb2 is read at batch slots via synthetic val=1
"edges" into the slot segment-sum. This cuts per-edge dma_gather
descriptor generation (the GPSIMD bottleneck) by ~2.1x.

Mechanics per layer are the baseline's: dest-row sharding, dma_gather of
source rows, VectorE scale + one-hot build, TensorE segment-sum into
PSUM (one accumulation group per PSUM bank), AllGather between layers.
Pad slots carry val=0 so garbage gathers are harmless.
"""

import sys

sys.path.insert(0, "/opt/trn_rl_repo")

import numpy as np
import ml_dtypes

import concourse.bacc as bacc
import concourse.bass as bass
import concourse.mybir as mybir
import concourse.tile as tile
from concourse.bass_utils import run_bass_kernel_spmd
from concourse import library_config

# ---------------------------------------------------------------- constants
N_USER = 100000
N_ITEM = 50000
N_NODES = 150000
N_EDGES = 2000000
EMB = 64
N_LAYERS = 3
BATCH = 2048
NC = 8

SHARD = N_NODES // NC            # 18750 logical rows per core
BLK = 128                        # dest rows per block (PSUM partitions)
NBLK = (SHARD + BLK - 1) // BLK  # 147 blocks per core
SHARD_PAD = NBLK * BLK           # 18816 physical rows per core
NPHYS = NC * SHARD_PAD           # 150528
NBANK = 5
BANKROWS = (NPHYS + NBANK - 1) // NBANK  # 30106 (< 32768 for int16)
SBLK = 16                        # blocks per superblock
NSB = (NBLK + SBLK - 1) // SBLK  # 10

OUT_ROWS = BATCH // NC           # 256 output user-rows per core
GSUB = 8                         # chunks per sub-gather (1024 indices)

F32 = mybir.dt.float32
BF16 = mybir.dt.bfloat16
I16 = mybir.dt.int16

_BF16NP = ml_dtypes.bfloat16


def _phys(node):
    """Global node id -> physical table row (per-core (p, blk) layout)."""
    node = np.asarray(node, dtype=np.int64)
    m = node // SHARD
    r = node - m * SHARD
    blk = r // BLK
    p = r - blk * BLK
    return m * SHARD_PAD + p * NBLK + blk


def _wrap_idx(seq):
    """Flat int16 index sequence -> [128, len/16] wrapped+replicated layout."""
    n = len(seq)
    assert n % 16 == 0
    w = np.asarray(seq, dtype=np.int16).reshape(-1, 16).T  # [16, n/16]
    return np.tile(w, (8, 1)).astype(np.int16)


def _prep_pass(dest_local, src_loc, src_bank, vals, dest_core, nblk, nbank, sblk):
    """Build per-core uniform chunk structure for one segment-sum pass.

    dest_local: local dest row within the owning core's range [0, nblk*128)
    src_loc:    source row within its bank (int, < 32768)
    src_bank:   source bank id
    vals:       edge values (float32)
    dest_core:  owning core of each edge
    Returns (meta, per-core arrays (idx16, valf, rbyte16)) where meta has
    gather groups [(slot_off, n_idx, bank)] and per-gather chunk lists
    [(blk, seg_jj)].
    """
    nseg = nblk * nbank
    counts = np.zeros((NC, nseg), dtype=np.int64)
    per_core = []
    for m in range(NC):
        sel = dest_core == m
        dl = dest_local[sel]
        blk = dl // BLK
        p = dl - blk * BLK
        key = blk * nbank + src_bank[sel]
        order = np.argsort(key, kind="stable")
        per_core.append(
            dict(key=key[order], loc=src_loc[sel][order], p=p[order], val=vals[sel][order])
        )
        counts[m] = np.bincount(key, minlength=nseg)

    cmax = counts.max(axis=0)
    C_seg = (cmax + BLK - 1) // BLK  # chunks per segment (uniform across cores)

    nsb = (nblk + sblk - 1) // sblk
    maxC = int(C_seg.max()) if len(C_seg) else 0
    # gather groups: one per (sb, bank); chunks within a group ordered by
    # (jj, blk) so emptier chunks sink to the gather tail (trailing -1 strip)
    gathers = []       # (slot_off, n_slots, bank)
    chunk_meta = []    # per gather: [(blk, jj), ...]
    # chunk_base_arr[seg, jj] -> slot offset of chunk jj of segment seg
    chunk_base_arr = np.full((nseg, max(maxC, 1)), -1, dtype=np.int64)
    pos = 0
    for sb in range(nsb):
        blks = list(range(sb * sblk, min((sb + 1) * sblk, nblk)))
        for b in range(nbank):
            meta = []
            maxjj = max((int(C_seg[blk * nbank + b]) for blk in blks), default=0)
            start = pos
            for jj in range(maxjj):
                for blk in blks:
                    if jj < C_seg[blk * nbank + b]:
                        meta.append((blk, jj))
                        chunk_base_arr[blk * nbank + b, jj] = pos
                        pos += BLK
            gathers.append((start, pos - start, b))
            chunk_meta.append(meta)
    S_total = pos

    core_arrays = []
    for m in range(NC):
        d = per_core[m]
        key = d["key"]
        nedge = len(key)
        first_of_key = np.zeros(nseg, dtype=np.int64)
        cnts = np.bincount(key, minlength=nseg)
        first_of_key[1:] = np.cumsum(cnts)[:-1]
        rank = np.arange(nedge) - first_of_key[key]
        jj = rank // BLK
        slot = chunk_base_arr[key, jj] + (rank - jj * BLK)
        assert (slot >= 0).all()

        idx16 = np.zeros(S_total, dtype=np.int16)
        valf = np.zeros(S_total, dtype=np.float32)
        rbyte = np.zeros(S_total, dtype=np.int16)
        idx16[slot] = d["loc"].astype(np.int16)
        valf[slot] = d["val"]
        rbyte[slot] = d["p"].astype(np.int16)
        core_arrays.append((idx16, valf, rbyte))

    meta = dict(
        gathers=gathers, chunk_meta=chunk_meta, S_total=S_total,
        nblk=nblk, nbank=nbank, sblk=sblk,
    )
    return meta, core_arrays


def _slot_cols(slot_arrays, S_total):
    """Stack per-core slot arrays into device input layouts."""
    outs = []
    for (idx16, valf, rbyte) in slot_arrays:
        idx_w = _wrap_idx(idx16)
        vals_t = valf.reshape(-1, BLK).T.copy()
        rbyte_t = rbyte.reshape(-1, BLK).T.astype(_BF16NP)
        outs.append((idx_w, vals_t, rbyte_t))
    return outs


def _prep_graph(adj_vals, adj_rows, adj_cols, users, items):
    rows = np.asarray(adj_rows, dtype=np.int64)
    cols = np.asarray(adj_cols, dtype=np.int64)
    vals = np.asarray(adj_vals, dtype=np.float32)
    users = np.asarray(users, dtype=np.int64)
    items = np.asarray(items, dtype=np.int64)

    t_nodes = np.concatenate([users, N_USER + items])  # [4096] with dups
    NSLOT = len(t_nodes)

    # ---- L3-proper edges: edges into t_nodes, replicated per slot
    t_order = np.argsort(t_nodes, kind="stable")
    t_sorted = t_nodes[t_order]
    lo = np.searchsorted(t_sorted, rows, side="left")
    hi = np.searchsorted(t_sorted, rows, side="right")
    nrep = hi - lo                       # slots per edge (mostly 0)
    esel = np.nonzero(nrep)[0]
    rep = nrep[esel]
    e_idx = np.repeat(esel, rep)         # edge index per L3 edge-instance
    # slot (sorted order) per instance
    starts = lo[esel]
    offs = np.arange(len(e_idx)) - np.repeat(
        np.concatenate([[0], np.cumsum(rep)[:-1]]), rep
    )
    slot3 = t_order[starts.repeat(rep) + offs]
    src3 = cols[e_idx]
    val3 = vals[e_idx]

    # ---- S2 = batch nodes + sources of L3 edges
    s2_mask = np.zeros(N_NODES, dtype=bool)
    s2_mask[t_nodes] = True
    s2_mask[src3] = True

    # ---- L2 edges: dest in S2
    sel2 = s2_mask[rows]
    rows2, cols2, vals2 = rows[sel2], cols[sel2], vals[sel2]

    # ---- slot sharding: slot -> home core of its node; local slot index
    slot_core = t_nodes // SHARD
    slot_local = np.zeros(NSLOT, dtype=np.int64)
    core_slot_count = np.zeros(NC, dtype=np.int64)
    for m in range(NC):
        sl = np.nonzero(slot_core == m)[0]
        slot_local[sl] = np.arange(len(sl))
        core_slot_count[m] = len(sl)
    SLOTP = int(-(-core_slot_count.max() // BLK) * BLK)
    NBLK3 = SLOTP // BLK

    # ---- pass structures
    sphys = _phys(cols)
    bank_all = np.minimum(sphys // BANKROWS, NBANK - 1)
    loc_all = sphys - bank_all * BANKROWS

    metaL1, arrL1 = _prep_pass(
        rows - (rows // SHARD) * SHARD,
        loc_all, bank_all, vals, rows // SHARD, NBLK, NBANK, SBLK,
    )
    sphys2 = _phys(cols2)
    bank2 = np.minimum(sphys2 // BANKROWS, NBANK - 1)
    loc2 = sphys2 - bank2 * BANKROWS
    metaL2, arrL2 = _prep_pass(
        rows2 - (rows2 // SHARD) * SHARD,
        loc2, bank2, vals2, rows2 // SHARD, NBLK, NBANK, SBLK,
    )
    # L3 proper: dest = local slot, src from full table2 (5 banks)
    sphys3 = _phys(src3)
    bank3 = np.minimum(sphys3 // BANKROWS, NBANK - 1)
    loc3 = sphys3 - bank3 * BANKROWS
    metaL3, arrL3 = _prep_pass(
        slot_local[slot3], loc3, bank3, val3, slot_core[slot3], NBLK3, NBANK, NBLK3,
    )
    # L3 synthetic: dest = local slot, src = LOCAL acc shard row, val = 1
    syn_slot = np.arange(NSLOT)
    syn_src_phys = _phys(t_nodes)                  # global phys row
    syn_loc = syn_src_phys - (syn_src_phys // SHARD_PAD) * SHARD_PAD  # local row
    metaSyn, arrSyn = _prep_pass(
        slot_local[syn_slot],
        syn_loc, np.zeros(NSLOT, dtype=np.int64), np.ones(NSLOT, dtype=np.float32),
        slot_core[syn_slot], NBLK3, 1, NBLK3,
    )

    # ---- final extraction indices
    def bounce_row(l):
        return (l % BLK) * NBLK3 + l // BLK

    exrow = slot_core * SLOTP + bounce_row(slot_local)  # ex_full row per slot
    exu = np.zeros((NC, OUT_ROWS), dtype=np.int16)
    for m in range(NC):
        exu[m] = exrow[m * OUT_ROWS : (m + 1) * OUT_ROWS].astype(np.int16)
    exi = exrow[BATCH:].astype(np.int16)  # item slots, same for all cores

    return dict(
        metaL1=metaL1, arrL1=arrL1,
        metaL2=metaL2, arrL2=arrL2,
        metaL3=metaL3, arrL3=arrL3,
        metaSyn=metaSyn, arrSyn=arrSyn,
        SLOTP=SLOTP, NBLK3=NBLK3, exu=exu, exi=exi,
    )


def _build(g):
    """Build the SPMD Bass graph (identical for all cores)."""
    metaL1, metaL2 = g["metaL1"], g["metaL2"]
    metaL3, metaSyn = g["metaL3"], g["metaSyn"]
    SLOTP, NBLK3 = g["SLOTP"], g["NBLK3"]
    S1, S2_, S3, SS = (
        metaL1["S_total"], metaL2["S_total"], metaL3["S_total"], metaSyn["S_total"],
    )

    nc = bacc.Bacc("TRN2", target_bir_lowering=False, num_swdge_queues=4)

    table0 = nc.declare_dram_parameter("table0", [NPHYS, EMB], F32, isOutput=False)
    acc0 = nc.declare_dram_parameter("acc0", [SHARD_PAD, EMB], F32, isOutput=False)
    idx1_in = nc.declare_dram_parameter("idx1", [128, S1 // 16], I16, isOutput=False)
    vals1_in = nc.declare_dram_parameter("vals1", [128, S1 // 128], F32, isOutput=False)
    rb1_in = nc.declare_dram_parameter("rb1", [128, S1 // 128], BF16, isOutput=False)
    idx2_in = nc.declare_dram_parameter("idx2", [128, S2_ // 16], I16, isOutput=False)
    vals2_in = nc.declare_dram_parameter("vals2", [128, S2_ // 128], F32, isOutput=False)
    rb2_in = nc.declare_dram_parameter("rb2", [128, S2_ // 128], BF16, isOutput=False)
    idx3_in = nc.declare_dram_parameter("idx3", [128, S3 // 16], I16, isOutput=False)
    vals3_in = nc.declare_dram_parameter("vals3", [128, S3 // 128], F32, isOutput=False)
    rb3_in = nc.declare_dram_parameter("rb3", [128, S3 // 128], BF16, isOutput=False)
    idxs_in = nc.declare_dram_parameter("idxs", [128, SS // 16], I16, isOutput=False)
    valss_in = nc.declare_dram_parameter("valss", [128, SS // 128], F32, isOutput=False)
    rbs_in = nc.declare_dram_parameter("rbs", [128, SS // 128], BF16, isOutput=False)
    iota_in = nc.declare_dram_parameter("iota", [128, 128], BF16, isOutput=False)
    ident_in = nc.declare_dram_parameter("ident", [128, 128], F32, isOutput=False)
    exu_in = nc.declare_dram_parameter("exu", [128, OUT_ROWS // 16], I16, isOutput=False)
    exi_in = nc.declare_dram_parameter("exi", [128, BATCH // 16], I16, isOutput=False)
    out_ext = nc.declare_dram_parameter("out", [OUT_ROWS, BATCH], F32, isOutput=True)

    with tile.TileContext(nc) as tc:
        nc.gpsimd.load_library(library_config.mlp)
        with (
            tc.tile_pool(name="const", bufs=1) as constp,
            tc.tile_pool(name="dram", bufs=1, space="DRAM") as dramp,
            tc.tile_pool(name="gpool", bufs=12) as gpool,
            tc.tile_pool(name="gspool", bufs=8) as gspool,
            tc.tile_pool(name="p01pool", bufs=8) as p01pool,
            tc.tile_pool(name="psum", bufs=2, space="PSUM") as psump,
            tc.tile_pool(name="fin", bufs=2) as finp,
            tc.tile_pool(name="fpsum", bufs=2, space="PSUM") as fpsump,
        ):
            tables = [table0]
            shard_bounces = []
            for l in range(2):
                sb_t = dramp.tile([SHARD_PAD, EMB], F32, name=f"shardb{l}")
                shard_bounces.append(sb_t)
                tb_t = dramp.tile([NPHYS, EMB], F32, addr_space="Shared", name=f"tableb{l + 1}")
                tables.append(tb_t)
            acc_dram = dramp.tile([SHARD_PAD, EMB], F32, name="acc_dram")
            ex_bounce = dramp.tile([SLOTP, EMB], F32, name="ex_bounce")
            ex_full = dramp.tile([NC * SLOTP, EMB], F32, addr_space="Shared", name="ex_full")

            idx1_sb = constp.tile([128, S1 // 16], I16)
            vals1_sb = constp.tile([128, S1 // 128], F32)
            rb1_sb = constp.tile([128, S1 // 128], BF16)
            idx2_sb = constp.tile([128, S2_ // 16], I16)
            vals2_sb = constp.tile([128, S2_ // 128], F32)
            rb2_sb = constp.tile([128, S2_ // 128], BF16)
            idx3_sb = constp.tile([128, S3 // 16], I16)
            vals3_sb = constp.tile([128, S3 // 128], F32)
            rb3_sb = constp.tile([128, S3 // 128], BF16)
            idxs_sb = constp.tile([128, SS // 16], I16)
            valss_sb = constp.tile([128, SS // 128], F32)
            rbs_sb = constp.tile([128, SS // 128], BF16)
            iota_sb = constp.tile([128, 128], BF16)
            ident_sb = constp.tile([128, 128], F32)
            acc_sb = constp.tile([128, NBLK * EMB], F32)
            dummy16 = constp.tile([128, 1], I16)
            dummyf = constp.tile([128, 1], F32)

            for sb_, in_ in (
                (idx1_sb, idx1_in), (vals1_sb, vals1_in), (rb1_sb, rb1_in),
                (idx2_sb, idx2_in), (vals2_sb, vals2_in), (rb2_sb, rb2_in),
                (idx3_sb, idx3_in), (vals3_sb, vals3_in), (rb3_sb, rb3_in),
                (idxs_sb, idxs_in), (valss_sb, valss_in), (rbs_sb, rbs_in),
                (iota_sb, iota_in), (ident_sb, ident_in),
            ):
                nc.sync.dma_start(sb_[:], in_[:])
            nc.sync.dma_start(
                acc_sb[:], acc0[:, :].rearrange("(p x) e -> p (x e)", p=128)
            )

            ghist = []

            def issue_gather(g_tile, col_off, nch_sub, src_ap, idx_slice):
                # reclaim window: wait the gather 10 back (gpool bufs=12, so
                # buffer reuse at i-12 is still covered: the guard at issue
                # i-2 directly waited DMA(i-12)) — deep so issue doesn't
                # stall on in-flight DMA latency
                if len(ghist) >= 10:
                    pt, po = ghist[-10]
                    nc.gpsimd.tensor_copy(out=dummyf[:, :1], in_=pt[:, po : po + 1])
                nc.gpsimd.memset(g_tile[:, col_off : col_off + 1], 0.0)
                nc.gpsimd.dma_gather(
                    out_ap=g_tile[
                        :, col_off : col_off + nch_sub * EMB
                    ].rearrange("p (c e) -> p c e", e=EMB),
                    in_ap=src_ap,
                    idxs_ap=idx_slice,
                    num_idxs=nch_sub * BLK,
                    num_idxs_reg=nch_sub * BLK,
                    elem_size=EMB,
                    queue_num=len(ghist) % 4,
                )
                ghist.append((g_tile, col_off))

            # absorbers for idx staging dependencies
            for t in (idx1_sb, idx2_sb, idx3_sb, idxs_sb):
                nc.gpsimd.tensor_copy(out=dummy16[:, :1], in_=t[:, :1])

            # pre-zero the gather buffers: trailing-stripped (-1) slots are
            # never written by the DMA, and 0 * garbage could be NaN
            for w in range(12):
                wt = gpool.tile([128, GSUB * EMB], F32, tag="g", name=f"gwarm_{w}")
                nc.vector.memset(wt[:], 0.0)

            def run_pass(meta, idx_sb, vals_sb, rb_sb, src_of_bank, psum_of_blk,
                         flags, lname):
                """Emit gathers + scale + one-hot + segment-sum matmuls.

                psum_of_blk(blk) -> (psum_tile, col); flags[(blk)] counts
                handled externally via `flags` dict {blk: [seen, total]}.
                """
                gathers, chunk_meta = meta["gathers"], meta["chunk_meta"]
                for gi, (off, n_idx, bank) in enumerate(gathers):
                    if n_idx == 0:
                        continue
                    nch = n_idx // BLK
                    src_ap = src_of_bank(bank)
                    nsub = (nch + GSUB - 1) // GSUB
                    for sg in range(nsub):
                        c_lo = sg * GSUB
                        nch_sub = min(GSUB, nch - c_lo)
                        goff = off + c_lo * BLK
                        gt = gpool.tile([128, GSUB * EMB], F32, tag="g", name=f"g_{lname}_{gi}_{sg}")
                        issue_gather(
                            gt, 0, nch_sub, src_ap,
                            idx_sb[:, goff // 16 : (goff + nch_sub * BLK) // 16],
                        )
                        gs = gspool.tile([128, GSUB * EMB], BF16, tag="gs", name=f"gs_{lname}_{gi}_{sg}")
                        c0 = goff // BLK
                        nc.vector.tensor_tensor(
                            out=gs[:, : nch_sub * EMB].rearrange("p (c e) -> p c e", e=EMB),
                            in0=gt[:, : nch_sub * EMB].rearrange("p (c e) -> p c e", e=EMB),
                            in1=vals_sb[:, c0 : c0 + nch_sub]
                            .rearrange("p (c o) -> p c o", o=1)
                            .to_broadcast([128, nch_sub, EMB]),
                            op=mybir.AluOpType.mult,
                        )
                        p01 = p01pool.tile([128, GSUB * 128], BF16, tag="p01", name=f"p01_{lname}_{gi}_{sg}")
                        nc.vector.tensor_tensor(
                            out=p01[:, : nch_sub * 128].rearrange("p (c q) -> p c q", q=128),
                            in0=rb_sb[:, c0 : c0 + nch_sub]
                            .rearrange("p (c o) -> p c o", o=1)
                            .to_broadcast([128, nch_sub, 128]),
                            in1=iota_sb[:, :]
                            .rearrange("p (o q) -> p o q", o=1)
                            .to_broadcast([128, nch_sub, 128]),
                            op=mybir.AluOpType.is_equal,
                        )
                        for jj_local in range(nch_sub):
                            j = c_lo + jj_local
                            blk, _jj = chunk_meta[gi][j]
                            ph, col, hkey = psum_of_blk(blk)
                            seen, total = flags[hkey]
                            nc.tensor.matmul(
                                out=ph[:, col * EMB : (col + 1) * EMB],
                                lhsT=p01[:, jj_local * 128 : (jj_local + 1) * 128],
                                rhs=gs[:, jj_local * EMB : (jj_local + 1) * EMB],
                                start=(seen == 0),
                                stop=(seen == total - 1),
                                skip_group_check=True,
                            )
                            flags[hkey][0] += 1

            # ================= L1 and L2 =================
            for l, (meta, isb, vsb, rsb) in enumerate((
                (metaL1, idx1_sb, vals1_sb, rb1_sb),
                (metaL2, idx2_sb, vals2_sb, rb2_sb),
            )):
                src = tables[l]
                gathers, chunk_meta = meta["gathers"], meta["chunk_meta"]
                ngather_per_sb = NBANK  # groups per superblock
                for sb in range(NSB):
                    blks = list(range(sb * SBLK, min((sb + 1) * SBLK, NBLK)))
                    nhalf = (len(blks) + 7) // 8
                    halves = [
                        psump.tile(
                            [128, min(8, len(blks) - 8 * h) * EMB], F32,
                            tag=f"ph{h}", name=f"ph_{l}_{sb}_{h}",
                        )
                        for h in range(nhalf)
                    ]
                    flags = {}
                    gsl = list(range(sb * ngather_per_sb, (sb + 1) * ngather_per_sb))
                    for gi in gsl:
                        for (blk, _jj) in chunk_meta[gi]:
                            h = (blk - sb * SBLK) // 8
                            flags.setdefault(h, [0, 0])[1] += 1

                    def psum_of_blk(blk, sb=sb, halves=halves):
                        h = (blk - sb * SBLK) // 8
                        return halves[h], (blk - sb * SBLK) % 8, h

                    sub_meta = dict(
                        gathers=[gathers[gi] for gi in gsl],
                        chunk_meta=[chunk_meta[gi] for gi in gsl],
                    )
                    run_pass(
                        sub_meta, isb, vsb, rsb,
                        lambda bank, src=src: src[
                            bank * BANKROWS : bank * BANKROWS + min(BANKROWS, NPHYS - bank * BANKROWS), :
                        ],
                        psum_of_blk, flags, f"l{l}s{sb}",
                    )
                    # drain superblock PSUM
                    for h, ph in enumerate(halves):
                        b0 = sb * SBLK + h * 8
                        nb = ph.shape[1] // EMB
                        if flags.get(h, [0, 0])[1] > 0:
                            nc.vector.tensor_tensor(
                                out=acc_sb[:, b0 * EMB : (b0 + nb) * EMB],
                                in0=acc_sb[:, b0 * EMB : (b0 + nb) * EMB],
                                in1=ph[:, :],
                                op=mybir.AluOpType.add,
                            )
                            lay = finp.tile([128, 8 * EMB], F32, tag="lay", name=f"lay_{l}_{sb}_{h}")
                            nc.scalar.copy(out=lay[:, : nb * EMB], in_=ph[:, :])
                            nc.sync.dma_start(
                                shard_bounces[l][:, :]
                                .rearrange("(p x) e -> p x e", p=128)[:, b0 : b0 + nb, :],
                                lay[:, : nb * EMB].rearrange("p (x e) -> p x e", e=EMB),
                            )
                nc.gpsimd.collective_compute(
                    "AllGather",
                    mybir.AluOpType.bypass,
                    ins=[shard_bounces[l][:, :].opt()],
                    outs=[tables[l + 1][:, :].opt()],
                    replica_groups=[list(range(NC))],
                )

            # write acc (= emb0+emb1+emb2 at this core's shard) for synthetic reads
            nc.sync.dma_start(
                acc_dram[:, :].rearrange("(p x) e -> p (x e)", p=128), acc_sb[:]
            )

            # ================= L3 slots (proper + synthetic) =================
            slot_psum = psump.tile([128, NBLK3 * EMB], F32, tag="ph0", name="slotp")
            nchunks3 = sum(len(c) for c in metaL3["chunk_meta"]) + sum(
                len(c) for c in metaSyn["chunk_meta"]
            )
            flags3 = {0: [0, nchunks3]}

            def psum_of_slot_blk(blk):
                return slot_psum, blk, 0

            run_pass(
                metaL3, idx3_sb, vals3_sb, rb3_sb,
                lambda bank: tables[2][
                    bank * BANKROWS : bank * BANKROWS + min(BANKROWS, NPHYS - bank * BANKROWS), :
                ],
                psum_of_slot_blk, flags3, "l3",
            )
            run_pass(
                metaSyn, idxs_sb, valss_sb, rbs_sb,
                lambda bank: acc_dram[:, :],
                psum_of_slot_blk, flags3, "syn",
            )

            # drain slot PSUM -> ex_bounce -> AllGather
            slot_sb = finp.tile([128, NBLK3 * EMB], F32, tag="slot_sb")
            nc.scalar.copy(out=slot_sb[:, :], in_=slot_psum[:, :])
            nc.sync.dma_start(
                ex_bounce[:, :].rearrange("(p x) e -> p (x e)", p=128), slot_sb[:]
            )
            nc.gpsimd.collective_compute(
                "AllGather",
                mybir.AluOpType.bypass,
                ins=[ex_bounce[:, :].opt()],
                outs=[ex_full[:, :].opt()],
                replica_groups=[list(range(NC))],
            )

            # ================= final extraction + GEMM =================
            exu_sb = finp.tile([128, OUT_ROWS // 16], I16, tag="exu")
            exi_sb = finp.tile([128, BATCH // 16], I16, tag="exi")
            nc.sync.dma_start(exu_sb[:], exu_in[:])
            nc.sync.dma_start(exi_sb[:], exi_in[:])
            nc.gpsimd.tensor_copy(out=dummy16[:, :1], in_=exu_sb[:, :1])
            nc.gpsimd.tensor_copy(out=dummy16[:, :1], in_=exi_sb[:, :1])

            u_sb = finp.tile([128, (OUT_ROWS // 128) * EMB], F32, tag="u")
            i_sb = finp.tile([128, (BATCH // 128) * EMB], F32, tag="i")
            issue_gather(u_sb, 0, OUT_ROWS // BLK, ex_full[:, :], exu_sb[:, :])
            for part in range(2):
                issue_gather(
                    i_sb, part * 8 * EMB, 8, ex_full[:, :],
                    exi_sb[:, part * 64 : (part + 1) * 64],
                )
            ut = finp.tile([64, (OUT_ROWS // 128) * 128], BF16, tag="ut")
            it = finp.tile([64, (BATCH // 128) * 128], BF16, tag="it")
            for t in range(OUT_ROWS // 128):
                tp = fpsump.tile([64, 128], F32, tag="tp", name=f"tpu_{t}")
                nc.tensor.transpose(out=tp[:, :], in_=u_sb[:, t * EMB : (t + 1) * EMB], identity=ident_sb[:, :])
                nc.vector.tensor_copy(out=ut[:, t * 128 : (t + 1) * 128], in_=tp[:, :])
            for t in range(BATCH // 128):
                tp = fpsump.tile([64, 128], F32, tag="tp", name=f"tpi_{t}")
                nc.tensor.transpose(out=tp[:, :], in_=i_sb[:, t * EMB : (t + 1) * EMB], identity=ident_sb[:, :])
                nc.vector.tensor_copy(out=it[:, t * 128 : (t + 1) * 128], in_=tp[:, :])
            for t in range(OUT_ROWS // 128):
                for q in range(BATCH // 512):
                    po = fpsump.tile([128, 512], F32, tag="po", name=f"po_{t}_{q}")
                    nc.tensor.matmul(
                        out=po[:, :],
                        lhsT=ut[:, t * 128 : (t + 1) * 128],
                        rhs=it[:, q * 512 : (q + 1) * 512],
                        start=True, stop=True,
                    )
                    ob = finp.tile([128, 512], F32, tag="ob", name=f"ob_{t}_{q}")
                    nc.scalar.activation(
                        out=ob[:, :], in_=po[:, :],
                        func=mybir.ActivationFunctionType.Sigmoid,
                        scale=1.0 / ((N_LAYERS + 1) ** 2),
                    )
                    nc.sync.dma_start(
                        out_ext[t * 128 : (t + 1) * 128, q * 512 : (q + 1) * 512],
                        ob[:, :],
                    )
    nc.compile()
    return nc


LAST_EXEC_NS = None
LAST_RES = None


def _ensure_trace_hook():
    """Install the axon NTFF profile hook if the image's antenv lacks it.

    Mirrors trn_agent_boot.trn_boot's step 6 (which degrades silently when
    antenv.axon_hooks is missing). Best-effort: any failure leaves tracing
    disabled, which run_bass_kernel_spmd already tolerates.
    """
    try:
        from antenv.axon_hooks import get_axon_ntff_profile_hook  # noqa: F401

        return  # real module present; boot already handled it
    except ImportError:
        pass
    try:
        import contextlib
        import ctypes
        import types

        import antenv

        lib = ctypes.CDLL("/opt/axon/libaxon_pjrt.so")
        if not hasattr(lib, "axon_start_nrt_profile"):
            return
        lib.axon_start_nrt_profile.argtypes = [
            ctypes.POINTER(ctypes.c_int64),
            ctypes.c_size_t,
        ]
        lib.axon_start_nrt_profile.restype = ctypes.c_int64
        lib.axon_stop_nrt_profile.argtypes = [ctypes.c_char_p]
        lib.axon_stop_nrt_profile.restype = ctypes.c_int64

        @contextlib.contextmanager
        def _hook(output_dir, device_ids):
            import jax

            jax.devices()
            if device_ids:
                ids = (ctypes.c_int64 * len(device_ids))(*device_ids)
                rc = lib.axon_start_nrt_profile(ids, len(device_ids))
            else:
                rc = lib.axon_start_nrt_profile(None, 0)
            if rc != 0:
                raise RuntimeError(f"axon_start_nrt_profile rc={rc}")
            try:
                yield
            finally:
                n = lib.axon_stop_nrt_profile(str(output_dir).encode())
                if n <= 0:
                    print(f"profile: {n} ntff files in {output_dir}")

        mod = types.ModuleType("antenv.axon_hooks")
        mod._hook = _hook
        mod.get_axon_ntff_profile_hook = lambda: mod._hook
        mod.set_axon_ntff_profile_hook = lambda h: setattr(mod, "_hook", h)
        sys.modules["antenv.axon_hooks"] = mod
        antenv.axon_hooks = mod
    except Exception:
        pass


def kernel(user_emb, item_emb, adj_vals, adj_rows, adj_cols, users, items):
    global LAST_EXEC_NS, LAST_RES
    user_emb = np.asarray(user_emb, dtype=np.float32)
    item_emb = np.asarray(item_emb, dtype=np.float32)

    g = _prep_graph(adj_vals, adj_rows, adj_cols, users, items)

    all_emb = np.concatenate([user_emb, item_emb], axis=0)
    table0 = np.zeros((NPHYS, EMB), dtype=np.float32)
    table0[_phys(np.arange(N_NODES))] = all_emb

    iota = np.tile(np.arange(128, dtype=_BF16NP)[None, :], (128, 1))
    ident = np.eye(128, dtype=np.float32)

    nc = _build(g)

    colsL1 = _slot_cols(g["arrL1"], g["metaL1"]["S_total"])
    colsL2 = _slot_cols(g["arrL2"], g["metaL2"]["S_total"])
    colsL3 = _slot_cols(g["arrL3"], g["metaL3"]["S_total"])
    colsSyn = _slot_cols(g["arrSyn"], g["metaSyn"]["S_total"])

    in_maps = []
    for m in range(NC):
        i1, v1, r1 = colsL1[m]
        i2, v2, r2 = colsL2[m]
        i3, v3, r3 = colsL3[m]
        isn, vsn, rsn = colsSyn[m]
        in_maps.append(
            {
                "table0": table0,
                "acc0": table0[m * SHARD_PAD : (m + 1) * SHARD_PAD],
                "idx1": i1, "vals1": v1, "rb1": r1,
                "idx2": i2, "vals2": v2, "rb2": r2,
                "idx3": i3, "vals3": v3, "rb3": r3,
                "idxs": isn, "valss": vsn, "rbs": rsn,
                "iota": iota, "ident": ident,
                "exu": _wrap_idx(g["exu"][m]),
                "exi": _wrap_idx(g["exi"]),
            }
        )

    _ensure_trace_hook()
    try:
        res = run_bass_kernel_spmd(nc, in_maps, core_ids=list(range(NC)), trace=True)
        LAST_EXEC_NS = res.exec_time_ns
    except Exception:
        res = run_bass_kernel_spmd(nc, in_maps, core_ids=list(range(NC)))
        LAST_EXEC_NS = None
    LAST_RES = res
    out = np.concatenate([res.results[m]["out"] for m in range(NC)], axis=0)
    return out.astype(np.float32)



# revision 9
# speedup vs baseline: 1.0566x; 1.0052x over previous
"""LightGCN message-passing kernel for 8 TRN2 NeuronCores — v2.

v2 adds backward pruning: the final output only needs light_out at the
4096 batch slots (users + items), so
  - L1 (emb1 = A@emb0) runs over all 2M edges (emb1 needed ~everywhere),
  - L2 (emb2 = A@emb1) only at S2 = batch nodes + their in-neighbors
    (~54K nodes, ~720K edges),
  - L3 (emb3 = A@emb2) only at the 4096 batch slots (~54K edges),
and acc = emb0+emb1+emb2 is read at batch slots via synthetic val=1
"edges" into the slot segment-sum. This cuts per-edge dma_gather
descriptor generation (the GPSIMD bottleneck) by ~2.1x.

Mechanics per layer are the baseline's: dest-row sharding, dma_gather of
source rows, VectorE scale + one-hot build, TensorE segment-sum into
PSUM (one accumulation group per PSUM bank), AllGather between layers.
Pad slots carry val=0 so garbage gathers are harmless.
"""

import sys

sys.path.insert(0, "/opt/trn_rl_repo")

import numpy as np
import ml_dtypes

import concourse.bacc as bacc
import concourse.bass as bass
import concourse.mybir as mybir
import concourse.tile as tile
from concourse.bass_utils import run_bass_kernel_spmd
from concourse import library_config

# ---------------------------------------------------------------- constants
N_USER = 100000
N_ITEM = 50000
N_NODES = 150000
N_EDGES = 2000000
EMB = 64
N_LAYERS = 3
BATCH = 2048
NC = 8

SHARD = N_NODES // NC            # 18750 logical rows per core
BLK = 128                        # dest rows per block (PSUM partitions)
NBLK = (SHARD + BLK - 1) // BLK  # 147 blocks per core
SHARD_PAD = NBLK * BLK           # 18816 physical rows per core
NPHYS = NC * SHARD_PAD           # 150528
NBANK = 5
BANKROWS = (NPHYS + NBANK - 1) // NBANK  # 30106 (< 32768 for int16)
SBLK = 16                        # blocks per superblock
NSB = (NBLK + SBLK - 1) // SBLK  # 10

OUT_ROWS = BATCH // NC           # 256 output user-rows per core
GSUB = 8                         # chunks per sub-gather (1024 indices)

F32 = mybir.dt.float32
BF16 = mybir.dt.bfloat16
I16 = mybir.dt.int16

_BF16NP = ml_dtypes.bfloat16


def _phys(node):
    """Global node id -> physical table row (per-core (p, blk) layout)."""
    node = np.asarray(node, dtype=np.int64)
    m = node // SHARD
    r = node - m * SHARD
    blk = r // BLK
    p = r - blk * BLK
    return m * SHARD_PAD + p * NBLK + blk


def _wrap_idx(seq):
    """Flat int16 index sequence -> [128, len/16] wrapped+replicated layout."""
    n = len(seq)
    assert n % 16 == 0
    w = np.asarray(seq, dtype=np.int16).reshape(-1, 16).T  # [16, n/16]
    return np.tile(w, (8, 1)).astype(np.int16)


def _prep_pass(dest_local, src_loc, src_bank, vals, dest_core, nblk, nbank, sblk):
    """Build per-core uniform chunk structure for one segment-sum pass.

    dest_local: local dest row within the owning core's range [0, nblk*128)
    src_loc:    source row within its bank (int, < 32768)
    src_bank:   source bank id
    vals:       edge values (float32)
    dest_core:  owning core of each edge
    Returns (meta, per-core arrays (idx16, valf, rbyte16)) where meta has
    gather groups [(slot_off, n_idx, bank)] and per-gather chunk lists
    [(blk, seg_jj)].
    """
    nseg = nblk * nbank
    counts = np.zeros((NC, nseg), dtype=np.int64)
    per_core = []
    for m in range(NC):
        sel = dest_core == m
        dl = dest_local[sel]
        blk = dl // BLK
        p = dl - blk * BLK
        key = blk * nbank + src_bank[sel]
        order = np.argsort(key, kind="stable")
        per_core.append(
            dict(key=key[order], loc=src_loc[sel][order], p=p[order], val=vals[sel][order])
        )
        counts[m] = np.bincount(key, minlength=nseg)

    cmax = counts.max(axis=0)
    C_seg = (cmax + BLK - 1) // BLK  # chunks per segment (uniform across cores)

    nsb = (nblk + sblk - 1) // sblk
    maxC = int(C_seg.max()) if len(C_seg) else 0
    # gather groups: one per (sb, bank); chunks within a group ordered by
    # (jj, blk) so emptier chunks sink to the gather tail (trailing -1 strip)
    gathers = []       # (slot_off, n_slots, bank)
    chunk_meta = []    # per gather: [(blk, jj), ...]
    # chunk_base_arr[seg, jj] -> slot offset of chunk jj of segment seg
    chunk_base_arr = np.full((nseg, max(maxC, 1)), -1, dtype=np.int64)
    pos = 0
    for sb in range(nsb):
        blks = list(range(sb * sblk, min((sb + 1) * sblk, nblk)))
        for b in range(nbank):
            meta = []
            maxjj = max((int(C_seg[blk * nbank + b]) for blk in blks), default=0)
            start = pos
            for jj in range(maxjj):
                for blk in blks:
                    if jj < C_seg[blk * nbank + b]:
                        meta.append((blk, jj))
                        chunk_base_arr[blk * nbank + b, jj] = pos
                        pos += BLK
            gathers.append((start, pos - start, b))
            chunk_meta.append(meta)
    S_total = pos

    core_arrays = []
    for m in range(NC):
        d = per_core[m]
        key = d["key"]
        nedge = len(key)
        first_of_key = np.zeros(nseg, dtype=np.int64)
        cnts = np.bincount(key, minlength=nseg)
        first_of_key[1:] = np.cumsum(cnts)[:-1]
        rank = np.arange(nedge) - first_of_key[key]
        jj = rank // BLK
        slot = chunk_base_arr[key, jj] + (rank - jj * BLK)
        assert (slot >= 0).all()

        idx16 = np.zeros(S_total, dtype=np.int16)
        valf = np.zeros(S_total, dtype=np.float32)
        rbyte = np.zeros(S_total, dtype=np.int16)
        idx16[slot] = d["loc"].astype(np.int16)
        valf[slot] = d["val"]
        rbyte[slot] = d["p"].astype(np.int16)
        core_arrays.append((idx16, valf, rbyte))

    meta = dict(
        gathers=gathers, chunk_meta=chunk_meta, S_total=S_total,
        nblk=nblk, nbank=nbank, sblk=sblk,
    )
    return meta, core_arrays


def _slot_cols(slot_arrays, S_total):
    """Stack per-core slot arrays into device input layouts."""
    outs = []
    for (idx16, valf, rbyte) in slot_arrays:
        idx_w = _wrap_idx(idx16)
        vals_t = valf.reshape(-1, BLK).T.copy()
        rbyte_t = rbyte.reshape(-1, BLK).T.astype(_BF16NP)
        outs.append((idx_w, vals_t, rbyte_t))
    return outs


def _prep_graph(adj_vals, adj_rows, adj_cols, users, items):
    rows = np.asarray(adj_rows, dtype=np.int64)
    cols = np.asarray(adj_cols, dtype=np.int64)
    vals = np.asarray(adj_vals, dtype=np.float32)
    users = np.asarray(users, dtype=np.int64)
    items = np.asarray(items, dtype=np.int64)

    t_nodes = np.concatenate([users, N_USER + items])  # [4096] with dups
    NSLOT = len(t_nodes)

    # ---- L3-proper edges: edges into t_nodes, replicated per slot
    t_order = np.argsort(t_nodes, kind="stable")
    t_sorted = t_nodes[t_order]
    lo = np.searchsorted(t_sorted, rows, side="left")
    hi = np.searchsorted(t_sorted, rows, side="right")
    nrep = hi - lo                       # slots per edge (mostly 0)
    esel = np.nonzero(nrep)[0]
    rep = nrep[esel]
    e_idx = np.repeat(esel, rep)         # edge index per L3 edge-instance
    # slot (sorted order) per instance
    starts = lo[esel]
    offs = np.arange(len(e_idx)) - np.repeat(
        np.concatenate([[0], np.cumsum(rep)[:-1]]), rep
    )
    slot3 = t_order[starts.repeat(rep) + offs]
    src3 = cols[e_idx]
    val3 = vals[e_idx]

    # ---- S2 = batch nodes + sources of L3 edges
    s2_mask = np.zeros(N_NODES, dtype=bool)
    s2_mask[t_nodes] = True
    s2_mask[src3] = True

    # ---- L2 edges: dest in S2
    sel2 = s2_mask[rows]
    rows2, cols2, vals2 = rows[sel2], cols[sel2], vals[sel2]

    # ---- slot sharding: slot -> home core of its node; local slot index
    slot_core = t_nodes // SHARD
    slot_local = np.zeros(NSLOT, dtype=np.int64)
    core_slot_count = np.zeros(NC, dtype=np.int64)
    for m in range(NC):
        sl = np.nonzero(slot_core == m)[0]
        slot_local[sl] = np.arange(len(sl))
        core_slot_count[m] = len(sl)
    SLOTP = int(-(-core_slot_count.max() // BLK) * BLK)
    NBLK3 = SLOTP // BLK

    # ---- pass structures
    sphys = _phys(cols)
    bank_all = np.minimum(sphys // BANKROWS, NBANK - 1)
    loc_all = sphys - bank_all * BANKROWS

    metaL1, arrL1 = _prep_pass(
        rows - (rows // SHARD) * SHARD,
        loc_all, bank_all, vals, rows // SHARD, NBLK, NBANK, SBLK,
    )
    sphys2 = _phys(cols2)
    bank2 = np.minimum(sphys2 // BANKROWS, NBANK - 1)
    loc2 = sphys2 - bank2 * BANKROWS
    metaL2, arrL2 = _prep_pass(
        rows2 - (rows2 // SHARD) * SHARD,
        loc2, bank2, vals2, rows2 // SHARD, NBLK, NBANK, SBLK,
    )
    # L3 proper: dest = local slot, src from full table2 (5 banks)
    sphys3 = _phys(src3)
    bank3 = np.minimum(sphys3 // BANKROWS, NBANK - 1)
    loc3 = sphys3 - bank3 * BANKROWS
    metaL3, arrL3 = _prep_pass(
        slot_local[slot3], loc3, bank3, val3, slot_core[slot3], NBLK3, NBANK, NBLK3,
    )
    # L3 synthetic: dest = local slot, src = LOCAL acc shard row, val = 1
    syn_slot = np.arange(NSLOT)
    syn_src_phys = _phys(t_nodes)                  # global phys row
    syn_loc = syn_src_phys - (syn_src_phys // SHARD_PAD) * SHARD_PAD  # local row
    metaSyn, arrSyn = _prep_pass(
        slot_local[syn_slot],
        syn_loc, np.zeros(NSLOT, dtype=np.int64), np.ones(NSLOT, dtype=np.float32),
        slot_core[syn_slot], NBLK3, 1, NBLK3,
    )

    # ---- final extraction indices
    def bounce_row(l):
        return (l % BLK) * NBLK3 + l // BLK

    exrow = slot_core * SLOTP + bounce_row(slot_local)  # ex_full row per slot
    exu = np.zeros((NC, OUT_ROWS), dtype=np.int16)
    for m in range(NC):
        exu[m] = exrow[m * OUT_ROWS : (m + 1) * OUT_ROWS].astype(np.int16)
    exi = exrow[BATCH:].astype(np.int16)  # item slots, same for all cores

    return dict(
        metaL1=metaL1, arrL1=arrL1,
        metaL2=metaL2, arrL2=arrL2,
        metaL3=metaL3, arrL3=arrL3,
        metaSyn=metaSyn, arrSyn=arrSyn,
        SLOTP=SLOTP, NBLK3=NBLK3, exu=exu, exi=exi,
    )


def _build(g):
    """Build the SPMD Bass graph (identical for all cores)."""
    metaL1, metaL2 = g["metaL1"], g["metaL2"]
    metaL3, metaSyn = g["metaL3"], g["metaSyn"]
    SLOTP, NBLK3 = g["SLOTP"], g["NBLK3"]
    S1, S2_, S3, SS = (
        metaL1["S_total"], metaL2["S_total"], metaL3["S_total"], metaSyn["S_total"],
    )

    nc = bacc.Bacc("TRN2", target_bir_lowering=False, num_swdge_queues=4)

    table0 = nc.declare_dram_parameter("table0", [NPHYS, EMB], F32, isOutput=False)
    acc0 = nc.declare_dram_parameter("acc0", [SHARD_PAD, EMB], F32, isOutput=False)
    idx1_in = nc.declare_dram_parameter("idx1", [128, S1 // 16], I16, isOutput=False)
    vals1_in = nc.declare_dram_parameter("vals1", [128, S1 // 128], F32, isOutput=False)
    rb1_in = nc.declare_dram_parameter("rb1", [128, S1 // 128], BF16, isOutput=False)
    idx2_in = nc.declare_dram_parameter("idx2", [128, S2_ // 16], I16, isOutput=False)
    vals2_in = nc.declare_dram_parameter("vals2", [128, S2_ // 128], F32, isOutput=False)
    rb2_in = nc.declare_dram_parameter("rb2", [128, S2_ // 128], BF16, isOutput=False)
    idx3_in = nc.declare_dram_parameter("idx3", [128, S3 // 16], I16, isOutput=False)
    vals3_in = nc.declare_dram_parameter("vals3", [128, S3 // 128], F32, isOutput=False)
    rb3_in = nc.declare_dram_parameter("rb3", [128, S3 // 128], BF16, isOutput=False)
    idxs_in = nc.declare_dram_parameter("idxs", [128, SS // 16], I16, isOutput=False)
    valss_in = nc.declare_dram_parameter("valss", [128, SS // 128], F32, isOutput=False)
    rbs_in = nc.declare_dram_parameter("rbs", [128, SS // 128], BF16, isOutput=False)
    iota_in = nc.declare_dram_parameter("iota", [128, 128], BF16, isOutput=False)
    ident_in = nc.declare_dram_parameter("ident", [128, 128], F32, isOutput=False)
    exu_in = nc.declare_dram_parameter("exu", [128, OUT_ROWS // 16], I16, isOutput=False)
    exi_in = nc.declare_dram_parameter("exi", [128, BATCH // 16], I16, isOutput=False)
    out_ext = nc.declare_dram_parameter("out", [OUT_ROWS, BATCH], F32, isOutput=True)

    with tile.TileContext(nc) as tc:
        nc.gpsimd.load_library(library_config.mlp)
        with (
            tc.tile_pool(name="const", bufs=1) as constp,
            tc.tile_pool(name="dram", bufs=1, space="DRAM") as dramp,
            tc.tile_pool(name="gpool", bufs=12) as gpool,
            tc.tile_pool(name="gspool", bufs=8) as gspool,
            tc.tile_pool(name="p01pool", bufs=8) as p01pool,
            tc.tile_pool(name="psum", bufs=2, space="PSUM") as psump,
            tc.tile_pool(name="fin", bufs=2) as finp,
            tc.tile_pool(name="fpsum", bufs=2, space="PSUM") as fpsump,
        ):
            tables = [table0]
            shard_bounces = []
            for l in range(2):
                sb_t = dramp.tile([SHARD_PAD, EMB], F32, name=f"shardb{l}")
                shard_bounces.append(sb_t)
                tb_t = dramp.tile([NPHYS, EMB], F32, addr_space="Shared", name=f"tableb{l + 1}")
                tables.append(tb_t)
            acc_dram = dramp.tile([SHARD_PAD, EMB], F32, name="acc_dram")
            ex_bounce = dramp.tile([SLOTP, EMB], F32, name="ex_bounce")
            ex_full = dramp.tile([NC * SLOTP, EMB], F32, addr_space="Shared", name="ex_full")

            idx1_sb = constp.tile([128, S1 // 16], I16)
            vals1_sb = constp.tile([128, S1 // 128], F32)
            rb1_sb = constp.tile([128, S1 // 128], BF16)
            idx2_sb = constp.tile([128, S2_ // 16], I16)
            vals2_sb = constp.tile([128, S2_ // 128], F32)
            rb2_sb = constp.tile([128, S2_ // 128], BF16)
            idx3_sb = constp.tile([128, S3 // 16], I16)
            vals3_sb = constp.tile([128, S3 // 128], F32)
            rb3_sb = constp.tile([128, S3 // 128], BF16)
            idxs_sb = constp.tile([128, SS // 16], I16)
            valss_sb = constp.tile([128, SS // 128], F32)
            rbs_sb = constp.tile([128, SS // 128], BF16)
            iota_sb = constp.tile([128, 128], BF16)
            ident_sb = constp.tile([128, 128], F32)
            acc_sb = constp.tile([128, NBLK * EMB], F32)
            dummy16 = constp.tile([128, 1], I16)
            dummyf = constp.tile([128, 1], F32)

            for sb_, in_ in (
                (idx1_sb, idx1_in), (vals1_sb, vals1_in), (rb1_sb, rb1_in),
                (idx2_sb, idx2_in), (vals2_sb, vals2_in), (rb2_sb, rb2_in),
                (idx3_sb, idx3_in), (vals3_sb, vals3_in), (rb3_sb, rb3_in),
                (idxs_sb, idxs_in), (valss_sb, valss_in), (rbs_sb, rbs_in),
                (iota_sb, iota_in), (ident_sb, ident_in),
            ):
                nc.sync.dma_start(sb_[:], in_[:])
            nc.sync.dma_start(
                acc_sb[:], acc0[:, :].rearrange("(p x) e -> p (x e)", p=128)
            )

            ghist = []
            # one register per distinct num_idxs value, written ONCE: a fresh
            # MOVE per gather into a shared register serializes each gather
            # behind the previous one's DMA-completion sem (reg WAR hazard)
            nidx_regs = {}

            def nidx_reg(v):
                if v not in nidx_regs:
                    nidx_regs[v] = nc.gpsimd.to_reg(v)
                return nidx_regs[v]

            def issue_gather(g_tile, col_off, nch_sub, src_ap, idx_slice):
                # reclaim window: wait the gather 10 back (gpool bufs=12, so
                # buffer reuse at i-12 is still covered: the guard at issue
                # i-2 directly waited DMA(i-12)) — deep so issue doesn't
                # stall on in-flight DMA latency
                if len(ghist) >= 10:
                    pt, po = ghist[-10]
                    nc.gpsimd.tensor_copy(out=dummyf[:, :1], in_=pt[:, po : po + 1])
                nc.gpsimd.memset(g_tile[:, col_off : col_off + 1], 0.0)
                nc.gpsimd.dma_gather(
                    out_ap=g_tile[
                        :, col_off : col_off + nch_sub * EMB
                    ].rearrange("p (c e) -> p c e", e=EMB),
                    in_ap=src_ap,
                    idxs_ap=idx_slice,
                    num_idxs=nch_sub * BLK,
                    num_idxs_reg=nidx_reg(nch_sub * BLK),
                    elem_size=EMB,
                    queue_num=len(ghist) % 4,
                )
                ghist.append((g_tile, col_off))

            # absorbers for idx staging dependencies
            for t in (idx1_sb, idx2_sb, idx3_sb, idxs_sb):
                nc.gpsimd.tensor_copy(out=dummy16[:, :1], in_=t[:, :1])

            # pre-zero the gather buffers: trailing-stripped (-1) slots are
            # never written by the DMA, and 0 * garbage could be NaN
            for w in range(12):
                wt = gpool.tile([128, GSUB * EMB], F32, tag="g", name=f"gwarm_{w}")
                nc.vector.memset(wt[:], 0.0)

            def run_pass(meta, idx_sb, vals_sb, rb_sb, src_of_bank, psum_of_blk,
                         flags, lname):
                """Emit gathers + scale + one-hot + segment-sum matmuls.

                psum_of_blk(blk) -> (psum_tile, col); flags[(blk)] counts
                handled externally via `flags` dict {blk: [seen, total]}.
                """
                gathers, chunk_meta = meta["gathers"], meta["chunk_meta"]
                for gi, (off, n_idx, bank) in enumerate(gathers):
                    if n_idx == 0:
                        continue
                    nch = n_idx // BLK
                    src_ap = src_of_bank(bank)
                    nsub = (nch + GSUB - 1) // GSUB
                    for sg in range(nsub):
                        c_lo = sg * GSUB
                        nch_sub = min(GSUB, nch - c_lo)
                        goff = off + c_lo * BLK
                        gt = gpool.tile([128, GSUB * EMB], F32, tag="g", name=f"g_{lname}_{gi}_{sg}")
                        issue_gather(
                            gt, 0, nch_sub, src_ap,
                            idx_sb[:, goff // 16 : (goff + nch_sub * BLK) // 16],
                        )
                        gs = gspool.tile([128, GSUB * EMB], BF16, tag="gs", name=f"gs_{lname}_{gi}_{sg}")
                        c0 = goff // BLK
                        nc.vector.tensor_tensor(
                            out=gs[:, : nch_sub * EMB].rearrange("p (c e) -> p c e", e=EMB),
                            in0=gt[:, : nch_sub * EMB].rearrange("p (c e) -> p c e", e=EMB),
                            in1=vals_sb[:, c0 : c0 + nch_sub]
                            .rearrange("p (c o) -> p c o", o=1)
                            .to_broadcast([128, nch_sub, EMB]),
                            op=mybir.AluOpType.mult,
                        )
                        p01 = p01pool.tile([128, GSUB * 128], BF16, tag="p01", name=f"p01_{lname}_{gi}_{sg}")
                        nc.vector.tensor_tensor(
                            out=p01[:, : nch_sub * 128].rearrange("p (c q) -> p c q", q=128),
                            in0=rb_sb[:, c0 : c0 + nch_sub]
                            .rearrange("p (c o) -> p c o", o=1)
                            .to_broadcast([128, nch_sub, 128]),
                            in1=iota_sb[:, :]
                            .rearrange("p (o q) -> p o q", o=1)
                            .to_broadcast([128, nch_sub, 128]),
                            op=mybir.AluOpType.is_equal,
                        )
                        for jj_local in range(nch_sub):
                            j = c_lo + jj_local
                            blk, _jj = chunk_meta[gi][j]
                            ph, col, hkey = psum_of_blk(blk)
                            seen, total = flags[hkey]
                            nc.tensor.matmul(
                                out=ph[:, col * EMB : (col + 1) * EMB],
                                lhsT=p01[:, jj_local * 128 : (jj_local + 1) * 128],
                                rhs=gs[:, jj_local * EMB : (jj_local + 1) * EMB],
                                start=(seen == 0),
                                stop=(seen == total - 1),
                                skip_group_check=True,
                            )
                            flags[hkey][0] += 1

            # ================= L1 and L2 =================
            for l, (meta, isb, vsb, rsb) in enumerate((
                (metaL1, idx1_sb, vals1_sb, rb1_sb),
                (metaL2, idx2_sb, vals2_sb, rb2_sb),
            )):
                src = tables[l]
                gathers, chunk_meta = meta["gathers"], meta["chunk_meta"]
                ngather_per_sb = NBANK  # groups per superblock
                for sb in range(NSB):
                    blks = list(range(sb * SBLK, min((sb + 1) * SBLK, NBLK)))
                    nhalf = (len(blks) + 7) // 8
                    halves = [
                        psump.tile(
                            [128, min(8, len(blks) - 8 * h) * EMB], F32,
                            tag=f"ph{h}", name=f"ph_{l}_{sb}_{h}",
                        )
                        for h in range(nhalf)
                    ]
                    flags = {}
                    gsl = list(range(sb * ngather_per_sb, (sb + 1) * ngather_per_sb))
                    for gi in gsl:
                        for (blk, _jj) in chunk_meta[gi]:
                            h = (blk - sb * SBLK) // 8
                            flags.setdefault(h, [0, 0])[1] += 1

                    def psum_of_blk(blk, sb=sb, halves=halves):
                        h = (blk - sb * SBLK) // 8
                        return halves[h], (blk - sb * SBLK) % 8, h

                    sub_meta = dict(
                        gathers=[gathers[gi] for gi in gsl],
                        chunk_meta=[chunk_meta[gi] for gi in gsl],
                    )
                    run_pass(
                        sub_meta, isb, vsb, rsb,
                        lambda bank, src=src: src[
                            bank * BANKROWS : bank * BANKROWS + min(BANKROWS, NPHYS - bank * BANKROWS), :
                        ],
                        psum_of_blk, flags, f"l{l}s{sb}",
                    )
                    # drain superblock PSUM
                    for h, ph in enumerate(halves):
                        b0 = sb * SBLK + h * 8
                        nb = ph.shape[1] // EMB
                        if flags.get(h, [0, 0])[1] > 0:
                            nc.vector.tensor_tensor(
                                out=acc_sb[:, b0 * EMB : (b0 + nb) * EMB],
                                in0=acc_sb[:, b0 * EMB : (b0 + nb) * EMB],
                                in1=ph[:, :],
                                op=mybir.AluOpType.add,
                            )
                            lay = finp.tile([128, 8 * EMB], F32, tag="lay", name=f"lay_{l}_{sb}_{h}")
                            nc.scalar.copy(out=lay[:, : nb * EMB], in_=ph[:, :])
                            nc.sync.dma_start(
                                shard_bounces[l][:, :]
                                .rearrange("(p x) e -> p x e", p=128)[:, b0 : b0 + nb, :],
                                lay[:, : nb * EMB].rearrange("p (x e) -> p x e", e=EMB),
                            )
                nc.gpsimd.collective_compute(
                    "AllGather",
                    mybir.AluOpType.bypass,
                    ins=[shard_bounces[l][:, :].opt()],
                    outs=[tables[l + 1][:, :].opt()],
                    replica_groups=[list(range(NC))],
                )

            # write acc (= emb0+emb1+emb2 at this core's shard) for synthetic reads
            nc.sync.dma_start(
                acc_dram[:, :].rearrange("(p x) e -> p (x e)", p=128), acc_sb[:]
            )

            # ================= L3 slots (proper + synthetic) =================
            slot_psum = psump.tile([128, NBLK3 * EMB], F32, tag="ph0", name="slotp")
            nchunks3 = sum(len(c) for c in metaL3["chunk_meta"]) + sum(
                len(c) for c in metaSyn["chunk_meta"]
            )
            flags3 = {0: [0, nchunks3]}

            def psum_of_slot_blk(blk):
                return slot_psum, blk, 0

            run_pass(
                metaL3, idx3_sb, vals3_sb, rb3_sb,
                lambda bank: tables[2][
                    bank * BANKROWS : bank * BANKROWS + min(BANKROWS, NPHYS - bank * BANKROWS), :
                ],
                psum_of_slot_blk, flags3, "l3",
            )
            run_pass(
                metaSyn, idxs_sb, valss_sb, rbs_sb,
                lambda bank: acc_dram[:, :],
                psum_of_slot_blk, flags3, "syn",
            )

            # drain slot PSUM -> ex_bounce -> AllGather
            slot_sb = finp.tile([128, NBLK3 * EMB], F32, tag="slot_sb")
            nc.scalar.copy(out=slot_sb[:, :], in_=slot_psum[:, :])
            nc.sync.dma_start(
                ex_bounce[:, :].rearrange("(p x) e -> p (x e)", p=128), slot_sb[:]
            )
            nc.gpsimd.collective_compute(
                "AllGather",
                mybir.AluOpType.bypass,
                ins=[ex_bounce[:, :].opt()],
                outs=[ex_full[:, :].opt()],
                replica_groups=[list(range(NC))],
            )

            # ================= final extraction + GEMM =================
            exu_sb = finp.tile([128, OUT_ROWS // 16], I16, tag="exu")
            exi_sb = finp.tile([128, BATCH // 16], I16, tag="exi")
            nc.sync.dma_start(exu_sb[:], exu_in[:])
            nc.sync.dma_start(exi_sb[:], exi_in[:])
            nc.gpsimd.tensor_copy(out=dummy16[:, :1], in_=exu_sb[:, :1])
            nc.gpsimd.tensor_copy(out=dummy16[:, :1], in_=exi_sb[:, :1])

            u_sb = finp.tile([128, (OUT_ROWS // 128) * EMB], F32, tag="u")
            i_sb = finp.tile([128, (BATCH // 128) * EMB], F32, tag="i")
            issue_gather(u_sb, 0, OUT_ROWS // BLK, ex_full[:, :], exu_sb[:, :])
            for part in range(2):
                issue_gather(
                    i_sb, part * 8 * EMB, 8, ex_full[:, :],
                    exi_sb[:, part * 64 : (part + 1) * 64],
                )
            ut = finp.tile([64, (OUT_ROWS // 128) * 128], BF16, tag="ut")
            it = finp.tile([64, (BATCH // 128) * 128], BF16, tag="it")
            for t in range(OUT_ROWS // 128):
                tp = fpsump.tile([64, 128], F32, tag="tp", name=f"tpu_{t}")
                nc.tensor.transpose(out=tp[:, :], in_=u_sb[:, t * EMB : (t + 1) * EMB], identity=ident_sb[:, :])
                nc.vector.tensor_copy(out=ut[:, t * 128 : (t + 1) * 128], in_=tp[:, :])
            for t in range(BATCH // 128):
                tp = fpsump.tile([64, 128], F32, tag="tp", name=f"tpi_{t}")
                nc.tensor.transpose(out=tp[:, :], in_=i_sb[:, t * EMB : (t + 1) * EMB], identity=ident_sb[:, :])
                nc.vector.tensor_copy(out=it[:, t * 128 : (t + 1) * 128], in_=tp[:, :])
            for t in range(OUT_ROWS // 128):
                for q in range(BATCH // 512):
                    po = fpsump.tile([128, 512], F32, tag="po", name=f"po_{t}_{q}")
                    nc.tensor.matmul(
                        out=po[:, :],
                        lhsT=ut[:, t * 128 : (t + 1) * 128],
                        rhs=it[:, q * 512 : (q + 1) * 512],
                        start=True, stop=True,
                    )
                    ob = finp.tile([128, 512], F32, tag="ob", name=f"ob_{t}_{q}")
                    nc.scalar.activation(
                        out=ob[:, :], in_=po[:, :],
                        func=mybir.ActivationFunctionType.Sigmoid,
                        scale=1.0 / ((N_LAYERS + 1) ** 2),
                    )
                    nc.sync.dma_start(
                        out_ext[t * 128 : (t + 1) * 128, q * 512 : (q + 1) * 512],
                        ob[:, :],
                    )
    nc.compile()
    return nc


LAST_EXEC_NS = None
LAST_RES = None


def _ensure_trace_hook():
    """Install the axon NTFF profile hook if the image's antenv lacks it.

    Mirrors trn_agent_boot.trn_boot's step 6 (which degrades silently when
    antenv.axon_hooks is missing). Best-effort: any failure leaves tracing
    disabled, which run_bass_kernel_spmd already tolerates.
    """
    try:
        from antenv.axon_hooks import get_axon_ntff_profile_hook  # noqa: F401

        return  # real module present; boot already handled it
    except ImportError:
        pass
    try:
        import contextlib
        import ctypes
        import types

        import antenv

        lib = ctypes.CDLL("/opt/axon/libaxon_pjrt.so")
        if not hasattr(lib, "axon_start_nrt_profile"):
            return
        lib.axon_start_nrt_profile.argtypes = [
            ctypes.POINTER(ctypes.c_int64),
            ctypes.c_size_t,
        ]
        lib.axon_start_nrt_profile.restype = ctypes.c_int64
        lib.axon_stop_nrt_profile.argtypes = [ctypes.c_char_p]
        lib.axon_stop_nrt_profile.restype = ctypes.c_int64

        @contextlib.contextmanager
        def _hook(output_dir, device_ids):
            import jax

            jax.devices()
            if device_ids:
                ids = (ctypes.c_int64 * len(device_ids))(*device_ids)
                rc = lib.axon_start_nrt_profile(ids, len(device_ids))
            else:
                rc = lib.axon_start_nrt_profile(None, 0)
            if rc != 0:
                raise RuntimeError(f"axon_start_nrt_profile rc={rc}")
            try:
                yield
            finally:
                n = lib.axon_stop_nrt_profile(str(output_dir).encode())
                if n <= 0:
                    print(f"profile: {n} ntff files in {output_dir}")

        mod = types.ModuleType("antenv.axon_hooks")
        mod._hook = _hook
        mod.get_axon_ntff_profile_hook = lambda: mod._hook
        mod.set_axon_ntff_profile_hook = lambda h: setattr(mod, "_hook", h)
        sys.modules["antenv.axon_hooks"] = mod
        antenv.axon_hooks = mod
    except Exception:
        pass


def kernel(user_emb, item_emb, adj_vals, adj_rows, adj_cols, users, items):
    global LAST_EXEC_NS, LAST_RES
    user_emb = np.asarray(user_emb, dtype=np.float32)
    item_emb = np.asarray(item_emb, dtype=np.float32)

    g = _prep_graph(adj_vals, adj_rows, adj_cols, users, items)

    all_emb = np.concatenate([user_emb, item_emb], axis=0)
    table0 = np.zeros((NPHYS, EMB), dtype=np.float32)
    table0[_phys(np.arange(N_NODES))] = all_emb

    iota = np.tile(np.arange(128, dtype=_BF16NP)[None, :], (128, 1))
    ident = np.eye(128, dtype=np.float32)

    nc = _build(g)

    colsL1 = _slot_cols(g["arrL1"], g["metaL1"]["S_total"])
    colsL2 = _slot_cols(g["arrL2"], g["metaL2"]["S_total"])
    colsL3 = _slot_cols(g["arrL3"], g["metaL3"]["S_total"])
    colsSyn = _slot_cols(g["arrSyn"], g["metaSyn"]["S_total"])

    in_maps = []
    for m in range(NC):
        i1, v1, r1 = colsL1[m]
        i2, v2, r2 = colsL2[m]
        i3, v3, r3 = colsL3[m]
        isn, vsn, rsn = colsSyn[m]
        in_maps.append(
            {
                "table0": table0,
                "acc0": table0[m * SHARD_PAD : (m + 1) * SHARD_PAD],
                "idx1": i1, "vals1": v1, "rb1": r1,
                "idx2": i2, "vals2": v2, "rb2": r2,
                "idx3": i3, "vals3": v3, "rb3": r3,
                "idxs": isn, "valss": vsn, "rbs": rsn,
                "iota": iota, "ident": ident,
                "exu": _wrap_idx(g["exu"][m]),
                "exi": _wrap_idx(g["exi"]),
            }
        )

    _ensure_trace_hook()
    try:
        res = run_bass_kernel_spmd(nc, in_maps, core_ids=list(range(NC)), trace=True)
        LAST_EXEC_NS = res.exec_time_ns
    except Exception:
        res = run_bass_kernel_spmd(nc, in_maps, core_ids=list(range(NC)))
        LAST_EXEC_NS = None
    LAST_RES = res
    out = np.concatenate([res.results[m]["out"] for m in range(NC)], axis=0)
    return out.astype(np.float32)



# revision 11
# speedup vs baseline: 1.0640x; 1.0070x over previous
"""LightGCN message-passing kernel for 8 TRN2 NeuronCores — v2.

v2 adds backward pruning: the final output only needs light_out at the
4096 batch slots (users + items), so
  - L1 (emb1 = A@emb0) runs over all 2M edges (emb1 needed ~everywhere),
  - L2 (emb2 = A@emb1) only at S2 = batch nodes + their in-neighbors
    (~54K nodes, ~720K edges),
  - L3 (emb3 = A@emb2) only at the 4096 batch slots (~54K edges),
and acc = emb0+emb1+emb2 is read at batch slots via synthetic val=1
"edges" into the slot segment-sum. This cuts per-edge dma_gather
descriptor generation (the GPSIMD bottleneck) by ~2.1x.

Mechanics per layer are the baseline's: dest-row sharding, dma_gather of
source rows, VectorE scale + one-hot build, TensorE segment-sum into
PSUM (one accumulation group per PSUM bank), AllGather between layers.
Pad slots carry val=0 so garbage gathers are harmless.
"""

import sys

sys.path.insert(0, "/opt/trn_rl_repo")

import numpy as np
import ml_dtypes

import concourse.bacc as bacc
import concourse.bass as bass
import concourse.mybir as mybir
import concourse.tile as tile
from concourse.bass_utils import run_bass_kernel_spmd
from concourse import library_config

# ---------------------------------------------------------------- constants
N_USER = 100000
N_ITEM = 50000
N_NODES = 150000
N_EDGES = 2000000
EMB = 64
N_LAYERS = 3
BATCH = 2048
NC = 8

SHARD = N_NODES // NC            # 18750 logical rows per core
BLK = 128                        # dest rows per block (PSUM partitions)
NBLK = (SHARD + BLK - 1) // BLK  # 147 blocks per core
SHARD_PAD = NBLK * BLK           # 18816 physical rows per core
NPHYS = NC * SHARD_PAD           # 150528
NBANK = 5
BANKROWS = (NPHYS + NBANK - 1) // NBANK  # 30106 (< 32768 for int16)
SBLK = 16                        # blocks per superblock
NSB = (NBLK + SBLK - 1) // SBLK  # 10

OUT_ROWS = BATCH // NC           # 256 output user-rows per core
GSUB = 8                         # chunks per sub-gather (1024 indices)

F32 = mybir.dt.float32
BF16 = mybir.dt.bfloat16
I16 = mybir.dt.int16

_BF16NP = ml_dtypes.bfloat16


def _phys(node):
    """Global node id -> physical table row (per-core (p, blk) layout)."""
    node = np.asarray(node, dtype=np.int64)
    m = node // SHARD
    r = node - m * SHARD
    blk = r // BLK
    p = r - blk * BLK
    return m * SHARD_PAD + p * NBLK + blk


def _wrap_idx(seq):
    """Flat int16 index sequence -> [128, len/16] wrapped+replicated layout."""
    n = len(seq)
    assert n % 16 == 0
    w = np.asarray(seq, dtype=np.int16).reshape(-1, 16).T  # [16, n/16]
    return np.tile(w, (8, 1)).astype(np.int16)


def _prep_pass(dest_local, src_loc, src_bank, vals, dest_core, nblk, nbank, sblk):
    """Build per-core uniform chunk structure for one segment-sum pass.

    dest_local: local dest row within the owning core's range [0, nblk*128)
    src_loc:    source row within its bank (int, < 32768)
    src_bank:   source bank id
    vals:       edge values (float32)
    dest_core:  owning core of each edge
    Returns (meta, per-core arrays (idx16, valf, rbyte16)) where meta has
    gather groups [(slot_off, n_idx, bank)] and per-gather chunk lists
    [(blk, seg_jj)].
    """
    nseg = nblk * nbank
    counts = np.zeros((NC, nseg), dtype=np.int64)
    per_core = []
    for m in range(NC):
        sel = dest_core == m
        dl = dest_local[sel]
        blk = dl // BLK
        p = dl - blk * BLK
        key = blk * nbank + src_bank[sel]
        order = np.argsort(key, kind="stable")
        per_core.append(
            dict(key=key[order], loc=src_loc[sel][order], p=p[order], val=vals[sel][order])
        )
        counts[m] = np.bincount(key, minlength=nseg)

    cmax = counts.max(axis=0)
    C_seg = (cmax + BLK - 1) // BLK  # chunks per segment (uniform across cores)

    nsb = (nblk + sblk - 1) // sblk
    maxC = int(C_seg.max()) if len(C_seg) else 0
    # gather groups: one per (sb, bank); chunks within a group ordered by
    # (jj, blk) so emptier chunks sink to the gather tail (trailing -1 strip)
    gathers = []       # (slot_off, n_slots, bank)
    chunk_meta = []    # per gather: [(blk, jj), ...]
    # chunk_base_arr[seg, jj] -> slot offset of chunk jj of segment seg
    chunk_base_arr = np.full((nseg, max(maxC, 1)), -1, dtype=np.int64)
    pos = 0
    for sb in range(nsb):
        blks = list(range(sb * sblk, min((sb + 1) * sblk, nblk)))
        for b in range(nbank):
            meta = []
            maxjj = max((int(C_seg[blk * nbank + b]) for blk in blks), default=0)
            start = pos
            for jj in range(maxjj):
                for blk in blks:
                    if jj < C_seg[blk * nbank + b]:
                        meta.append((blk, jj))
                        chunk_base_arr[blk * nbank + b, jj] = pos
                        pos += BLK
            gathers.append((start, pos - start, b))
            chunk_meta.append(meta)
    S_total = pos

    core_arrays = []
    for m in range(NC):
        d = per_core[m]
        key = d["key"]
        nedge = len(key)
        first_of_key = np.zeros(nseg, dtype=np.int64)
        cnts = np.bincount(key, minlength=nseg)
        first_of_key[1:] = np.cumsum(cnts)[:-1]
        rank = np.arange(nedge) - first_of_key[key]
        jj = rank // BLK
        slot = chunk_base_arr[key, jj] + (rank - jj * BLK)
        assert (slot >= 0).all()

        idx16 = np.zeros(S_total, dtype=np.int16)
        valf = np.zeros(S_total, dtype=np.float32)
        rbyte = np.zeros(S_total, dtype=np.int16)
        idx16[slot] = d["loc"].astype(np.int16)
        valf[slot] = d["val"]
        rbyte[slot] = d["p"].astype(np.int16)
        core_arrays.append((idx16, valf, rbyte))

    meta = dict(
        gathers=gathers, chunk_meta=chunk_meta, S_total=S_total,
        nblk=nblk, nbank=nbank, sblk=sblk,
    )
    return meta, core_arrays


def _slot_cols(slot_arrays, S_total):
    """Stack per-core slot arrays into device input layouts."""
    outs = []
    for (idx16, valf, rbyte) in slot_arrays:
        idx_w = _wrap_idx(idx16)
        vals_t = valf.reshape(-1, BLK).T.copy()
        rbyte_t = rbyte.reshape(-1, BLK).T.astype(_BF16NP)
        outs.append((idx_w, vals_t, rbyte_t))
    return outs


def _prep_graph(adj_vals, adj_rows, adj_cols, users, items):
    rows = np.asarray(adj_rows, dtype=np.int64)
    cols = np.asarray(adj_cols, dtype=np.int64)
    vals = np.asarray(adj_vals, dtype=np.float32)
    users = np.asarray(users, dtype=np.int64)
    items = np.asarray(items, dtype=np.int64)

    t_nodes = np.concatenate([users, N_USER + items])  # [4096] with dups
    NSLOT = len(t_nodes)

    # ---- L3-proper edges: edges into t_nodes, replicated per slot
    t_order = np.argsort(t_nodes, kind="stable")
    t_sorted = t_nodes[t_order]
    lo = np.searchsorted(t_sorted, rows, side="left")
    hi = np.searchsorted(t_sorted, rows, side="right")
    nrep = hi - lo                       # slots per edge (mostly 0)
    esel = np.nonzero(nrep)[0]
    rep = nrep[esel]
    e_idx = np.repeat(esel, rep)         # edge index per L3 edge-instance
    # slot (sorted order) per instance
    starts = lo[esel]
    offs = np.arange(len(e_idx)) - np.repeat(
        np.concatenate([[0], np.cumsum(rep)[:-1]]), rep
    )
    slot3 = t_order[starts.repeat(rep) + offs]
    src3 = cols[e_idx]
    val3 = vals[e_idx]

    # ---- S2 = batch nodes + sources of L3 edges
    s2_mask = np.zeros(N_NODES, dtype=bool)
    s2_mask[t_nodes] = True
    s2_mask[src3] = True

    # ---- L2 edges: dest in S2
    sel2 = s2_mask[rows]
    rows2, cols2, vals2 = rows[sel2], cols[sel2], vals[sel2]

    # ---- slot sharding: slot -> home core of its node; local slot index
    slot_core = t_nodes // SHARD
    slot_local = np.zeros(NSLOT, dtype=np.int64)
    core_slot_count = np.zeros(NC, dtype=np.int64)
    for m in range(NC):
        sl = np.nonzero(slot_core == m)[0]
        slot_local[sl] = np.arange(len(sl))
        core_slot_count[m] = len(sl)
    SLOTP = int(-(-core_slot_count.max() // BLK) * BLK)
    NBLK3 = SLOTP // BLK

    # ---- pass structures
    sphys = _phys(cols)
    bank_all = np.minimum(sphys // BANKROWS, NBANK - 1)
    loc_all = sphys - bank_all * BANKROWS

    metaL1, arrL1 = _prep_pass(
        rows - (rows // SHARD) * SHARD,
        loc_all, bank_all, vals, rows // SHARD, NBLK, NBANK, SBLK,
    )
    sphys2 = _phys(cols2)
    bank2 = np.minimum(sphys2 // BANKROWS, NBANK - 1)
    loc2 = sphys2 - bank2 * BANKROWS
    metaL2, arrL2 = _prep_pass(
        rows2 - (rows2 // SHARD) * SHARD,
        loc2, bank2, vals2, rows2 // SHARD, NBLK, NBANK, SBLK,
    )
    # L3 proper: dest = local slot, src from full table2 (5 banks)
    sphys3 = _phys(src3)
    bank3 = np.minimum(sphys3 // BANKROWS, NBANK - 1)
    loc3 = sphys3 - bank3 * BANKROWS
    metaL3, arrL3 = _prep_pass(
        slot_local[slot3], loc3, bank3, val3, slot_core[slot3], NBLK3, NBANK, NBLK3,
    )
    # L3 synthetic: dest = local slot, src = LOCAL acc shard row, val = 1
    syn_slot = np.arange(NSLOT)
    syn_src_phys = _phys(t_nodes)                  # global phys row
    syn_loc = syn_src_phys - (syn_src_phys // SHARD_PAD) * SHARD_PAD  # local row
    metaSyn, arrSyn = _prep_pass(
        slot_local[syn_slot],
        syn_loc, np.zeros(NSLOT, dtype=np.int64), np.ones(NSLOT, dtype=np.float32),
        slot_core[syn_slot], NBLK3, 1, NBLK3,
    )

    # ---- final extraction indices
    def bounce_row(l):
        return (l % BLK) * NBLK3 + l // BLK

    exrow = slot_core * SLOTP + bounce_row(slot_local)  # ex_full row per slot
    exu = np.zeros((NC, OUT_ROWS), dtype=np.int16)
    for m in range(NC):
        exu[m] = exrow[m * OUT_ROWS : (m + 1) * OUT_ROWS].astype(np.int16)
    exi = exrow[BATCH:].astype(np.int16)  # item slots, same for all cores

    return dict(
        metaL1=metaL1, arrL1=arrL1,
        metaL2=metaL2, arrL2=arrL2,
        metaL3=metaL3, arrL3=arrL3,
        metaSyn=metaSyn, arrSyn=arrSyn,
        SLOTP=SLOTP, NBLK3=NBLK3, exu=exu, exi=exi,
    )


def _build(g):
    """Build the SPMD Bass graph (identical for all cores)."""
    metaL1, metaL2 = g["metaL1"], g["metaL2"]
    metaL3, metaSyn = g["metaL3"], g["metaSyn"]
    SLOTP, NBLK3 = g["SLOTP"], g["NBLK3"]
    S1, S2_, S3, SS = (
        metaL1["S_total"], metaL2["S_total"], metaL3["S_total"], metaSyn["S_total"],
    )

    nc = bacc.Bacc("TRN2", target_bir_lowering=False, num_swdge_queues=4)

    table0 = nc.declare_dram_parameter("table0", [NPHYS, EMB], F32, isOutput=False)
    acc0 = nc.declare_dram_parameter("acc0", [SHARD_PAD, EMB], F32, isOutput=False)
    idx1_in = nc.declare_dram_parameter("idx1", [128, S1 // 16], I16, isOutput=False)
    vals1_in = nc.declare_dram_parameter("vals1", [128, S1 // 128], F32, isOutput=False)
    rb1_in = nc.declare_dram_parameter("rb1", [128, S1 // 128], BF16, isOutput=False)
    idx2_in = nc.declare_dram_parameter("idx2", [128, S2_ // 16], I16, isOutput=False)
    vals2_in = nc.declare_dram_parameter("vals2", [128, S2_ // 128], F32, isOutput=False)
    rb2_in = nc.declare_dram_parameter("rb2", [128, S2_ // 128], BF16, isOutput=False)
    idx3_in = nc.declare_dram_parameter("idx3", [128, S3 // 16], I16, isOutput=False)
    vals3_in = nc.declare_dram_parameter("vals3", [128, S3 // 128], F32, isOutput=False)
    rb3_in = nc.declare_dram_parameter("rb3", [128, S3 // 128], BF16, isOutput=False)
    idxs_in = nc.declare_dram_parameter("idxs", [128, SS // 16], I16, isOutput=False)
    valss_in = nc.declare_dram_parameter("valss", [128, SS // 128], F32, isOutput=False)
    rbs_in = nc.declare_dram_parameter("rbs", [128, SS // 128], BF16, isOutput=False)
    iota_in = nc.declare_dram_parameter("iota", [128, 128], BF16, isOutput=False)
    ident_in = nc.declare_dram_parameter("ident", [128, 128], F32, isOutput=False)
    exu_in = nc.declare_dram_parameter("exu", [128, OUT_ROWS // 16], I16, isOutput=False)
    exi_in = nc.declare_dram_parameter("exi", [128, BATCH // 16], I16, isOutput=False)
    out_ext = nc.declare_dram_parameter("out", [OUT_ROWS, BATCH], F32, isOutput=True)

    with tile.TileContext(nc) as tc:
        nc.gpsimd.load_library(library_config.mlp)
        with (
            tc.tile_pool(name="const", bufs=1) as constp,
            tc.tile_pool(name="dram", bufs=1, space="DRAM") as dramp,
            tc.tile_pool(name="gpool", bufs=19) as gpool,
            tc.tile_pool(name="gspool", bufs=8) as gspool,
            tc.tile_pool(name="p01pool", bufs=8) as p01pool,
            tc.tile_pool(name="psum", bufs=2, space="PSUM") as psump,
            tc.tile_pool(name="fin", bufs=2) as finp,
            tc.tile_pool(name="fpsum", bufs=2, space="PSUM") as fpsump,
        ):
            tables = [table0]
            shard_bounces = []
            for l in range(2):
                sb_t = dramp.tile([SHARD_PAD, EMB], F32, name=f"shardb{l}")
                shard_bounces.append(sb_t)
                tb_t = dramp.tile([NPHYS, EMB], F32, addr_space="Shared", name=f"tableb{l + 1}")
                tables.append(tb_t)
            acc_dram = dramp.tile([SHARD_PAD, EMB], F32, name="acc_dram")
            ex_bounce = dramp.tile([SLOTP, EMB], F32, name="ex_bounce")
            ex_full = dramp.tile([NC * SLOTP, EMB], F32, addr_space="Shared", name="ex_full")

            idx1_sb = constp.tile([128, S1 // 16], I16)
            vals1_sb = constp.tile([128, S1 // 128], F32)
            rb1_sb = constp.tile([128, S1 // 128], BF16)
            idx2_sb = constp.tile([128, S2_ // 16], I16)
            vals2_sb = constp.tile([128, S2_ // 128], F32)
            rb2_sb = constp.tile([128, S2_ // 128], BF16)
            idx3_sb = constp.tile([128, S3 // 16], I16)
            vals3_sb = constp.tile([128, S3 // 128], F32)
            rb3_sb = constp.tile([128, S3 // 128], BF16)
            idxs_sb = constp.tile([128, SS // 16], I16)
            valss_sb = constp.tile([128, SS // 128], F32)
            rbs_sb = constp.tile([128, SS // 128], BF16)
            iota_sb = constp.tile([128, 128], BF16)
            ident_sb = constp.tile([128, 128], F32)
            acc_sb = constp.tile([128, NBLK * EMB], F32)
            dummy16 = constp.tile([128, 1], I16)
            dummyf = constp.tile([128, 1], F32)

            for sb_, in_ in (
                (idx1_sb, idx1_in), (vals1_sb, vals1_in), (rb1_sb, rb1_in),
                (idx2_sb, idx2_in), (vals2_sb, vals2_in), (rb2_sb, rb2_in),
                (idx3_sb, idx3_in), (vals3_sb, vals3_in), (rb3_sb, rb3_in),
                (idxs_sb, idxs_in), (valss_sb, valss_in), (rbs_sb, rbs_in),
                (iota_sb, iota_in), (ident_sb, ident_in),
            ):
                nc.sync.dma_start(sb_[:], in_[:])
            nc.sync.dma_start(
                acc_sb[:], acc0[:, :].rearrange("(p x) e -> p (x e)", p=128)
            )

            ghist = []
            # one register per distinct num_idxs value, written ONCE: a fresh
            # MOVE per gather into a shared register serializes each gather
            # behind the previous one's DMA-completion sem (reg WAR hazard)
            nidx_regs = {}

            def nidx_reg(v):
                if v not in nidx_regs:
                    nidx_regs[v] = nc.gpsimd.to_reg(v)
                return nidx_regs[v]

            def issue_gather(g_tile, col_off, nch_sub, src_ap, idx_slice):
                # reclaim window: wait the gather 17 back (gpool bufs=19, so
                # buffer reuse at i-19 is still covered: the guard at issue
                # i-2 directly waited DMA(i-19)) — deep because DMA completion
                # tails reach ~70us when gather packets queue behind static/CC
                # DMA on the shared engines
                if len(ghist) >= 17:
                    pt, po = ghist[-17]
                    nc.gpsimd.tensor_copy(out=dummyf[:, :1], in_=pt[:, po : po + 1])
                nc.gpsimd.memset(g_tile[:, col_off : col_off + 1], 0.0)
                nc.gpsimd.dma_gather(
                    out_ap=g_tile[
                        :, col_off : col_off + nch_sub * EMB
                    ].rearrange("p (c e) -> p c e", e=EMB),
                    in_ap=src_ap,
                    idxs_ap=idx_slice,
                    num_idxs=nch_sub * BLK,
                    num_idxs_reg=nidx_reg(nch_sub * BLK),
                    elem_size=EMB,
                    queue_num=len(ghist) % 4,
                )
                ghist.append((g_tile, col_off))

            # absorbers for idx staging dependencies
            for t in (idx1_sb, idx2_sb, idx3_sb, idxs_sb):
                nc.gpsimd.tensor_copy(out=dummy16[:, :1], in_=t[:, :1])

            # pre-zero the gather buffers: trailing-stripped (-1) slots are
            # never written by the DMA, and 0 * garbage could be NaN
            for w in range(19):
                wt = gpool.tile([128, GSUB * EMB], F32, tag="g", name=f"gwarm_{w}")
                nc.vector.memset(wt[:], 0.0)

            def run_pass(meta, idx_sb, vals_sb, rb_sb, src_of_bank, psum_of_blk,
                         flags, lname):
                """Emit gathers + scale + one-hot + segment-sum matmuls.

                psum_of_blk(blk) -> (psum_tile, col); flags[(blk)] counts
                handled externally via `flags` dict {blk: [seen, total]}.
                """
                gathers, chunk_meta = meta["gathers"], meta["chunk_meta"]
                for gi, (off, n_idx, bank) in enumerate(gathers):
                    if n_idx == 0:
                        continue
                    nch = n_idx // BLK
                    src_ap = src_of_bank(bank)
                    nsub = (nch + GSUB - 1) // GSUB
                    for sg in range(nsub):
                        c_lo = sg * GSUB
                        nch_sub = min(GSUB, nch - c_lo)
                        goff = off + c_lo * BLK
                        gt = gpool.tile([128, GSUB * EMB], F32, tag="g", name=f"g_{lname}_{gi}_{sg}")
                        issue_gather(
                            gt, 0, nch_sub, src_ap,
                            idx_sb[:, goff // 16 : (goff + nch_sub * BLK) // 16],
                        )
                        gs = gspool.tile([128, GSUB * EMB], BF16, tag="gs", name=f"gs_{lname}_{gi}_{sg}")
                        c0 = goff // BLK
                        nc.vector.tensor_tensor(
                            out=gs[:, : nch_sub * EMB].rearrange("p (c e) -> p c e", e=EMB),
                            in0=gt[:, : nch_sub * EMB].rearrange("p (c e) -> p c e", e=EMB),
                            in1=vals_sb[:, c0 : c0 + nch_sub]
                            .rearrange("p (c o) -> p c o", o=1)
                            .to_broadcast([128, nch_sub, EMB]),
                            op=mybir.AluOpType.mult,
                        )
                        p01 = p01pool.tile([128, GSUB * 128], BF16, tag="p01", name=f"p01_{lname}_{gi}_{sg}")
                        nc.vector.tensor_tensor(
                            out=p01[:, : nch_sub * 128].rearrange("p (c q) -> p c q", q=128),
                            in0=rb_sb[:, c0 : c0 + nch_sub]
                            .rearrange("p (c o) -> p c o", o=1)
                            .to_broadcast([128, nch_sub, 128]),
                            in1=iota_sb[:, :]
                            .rearrange("p (o q) -> p o q", o=1)
                            .to_broadcast([128, nch_sub, 128]),
                            op=mybir.AluOpType.is_equal,
                        )
                        for jj_local in range(nch_sub):
                            j = c_lo + jj_local
                            blk, _jj = chunk_meta[gi][j]
                            ph, col, hkey = psum_of_blk(blk)
                            seen, total = flags[hkey]
                            nc.tensor.matmul(
                                out=ph[:, col * EMB : (col + 1) * EMB],
                                lhsT=p01[:, jj_local * 128 : (jj_local + 1) * 128],
                                rhs=gs[:, jj_local * EMB : (jj_local + 1) * EMB],
                                start=(seen == 0),
                                stop=(seen == total - 1),
                                skip_group_check=True,
                            )
                            flags[hkey][0] += 1

            # ================= L1 and L2 =================
            for l, (meta, isb, vsb, rsb) in enumerate((
                (metaL1, idx1_sb, vals1_sb, rb1_sb),
                (metaL2, idx2_sb, vals2_sb, rb2_sb),
            )):
                src = tables[l]
                gathers, chunk_meta = meta["gathers"], meta["chunk_meta"]
                ngather_per_sb = NBANK  # groups per superblock
                for sb in range(NSB):
                    blks = list(range(sb * SBLK, min((sb + 1) * SBLK, NBLK)))
                    nhalf = (len(blks) + 7) // 8
                    halves = [
                        psump.tile(
                            [128, min(8, len(blks) - 8 * h) * EMB], F32,
                            tag=f"ph{h}", name=f"ph_{l}_{sb}_{h}",
                        )
                        for h in range(nhalf)
                    ]
                    flags = {}
                    gsl = list(range(sb * ngather_per_sb, (sb + 1) * ngather_per_sb))
                    for gi in gsl:
                        for (blk, _jj) in chunk_meta[gi]:
                            h = (blk - sb * SBLK) // 8
                            flags.setdefault(h, [0, 0])[1] += 1

                    def psum_of_blk(blk, sb=sb, halves=halves):
                        h = (blk - sb * SBLK) // 8
                        return halves[h], (blk - sb * SBLK) % 8, h

                    sub_meta = dict(
                        gathers=[gathers[gi] for gi in gsl],
                        chunk_meta=[chunk_meta[gi] for gi in gsl],
                    )
                    run_pass(
                        sub_meta, isb, vsb, rsb,
                        lambda bank, src=src: src[
                            bank * BANKROWS : bank * BANKROWS + min(BANKROWS, NPHYS - bank * BANKROWS), :
                        ],
                        psum_of_blk, flags, f"l{l}s{sb}",
                    )
                    # drain superblock PSUM
                    for h, ph in enumerate(halves):
                        b0 = sb * SBLK + h * 8
                        nb = ph.shape[1] // EMB
                        if flags.get(h, [0, 0])[1] > 0:
                            nc.vector.tensor_tensor(
                                out=acc_sb[:, b0 * EMB : (b0 + nb) * EMB],
                                in0=acc_sb[:, b0 * EMB : (b0 + nb) * EMB],
                                in1=ph[:, :],
                                op=mybir.AluOpType.add,
                            )
                            lay = finp.tile([128, 8 * EMB], F32, tag="lay", name=f"lay_{l}_{sb}_{h}")
                            nc.scalar.copy(out=lay[:, : nb * EMB], in_=ph[:, :])
                            nc.sync.dma_start(
                                shard_bounces[l][:, :]
                                .rearrange("(p x) e -> p x e", p=128)[:, b0 : b0 + nb, :],
                                lay[:, : nb * EMB].rearrange("p (x e) -> p x e", e=EMB),
                            )
                nc.gpsimd.collective_compute(
                    "AllGather",
                    mybir.AluOpType.bypass,
                    ins=[shard_bounces[l][:, :].opt()],
                    outs=[tables[l + 1][:, :].opt()],
                    replica_groups=[list(range(NC))],
                )

            # write acc (= emb0+emb1+emb2 at this core's shard) for synthetic reads
            nc.sync.dma_start(
                acc_dram[:, :].rearrange("(p x) e -> p (x e)", p=128), acc_sb[:]
            )

            # ================= L3 slots (proper + synthetic) =================
            slot_psum = psump.tile([128, NBLK3 * EMB], F32, tag="ph0", name="slotp")
            nchunks3 = sum(len(c) for c in metaL3["chunk_meta"]) + sum(
                len(c) for c in metaSyn["chunk_meta"]
            )
            flags3 = {0: [0, nchunks3]}

            def psum_of_slot_blk(blk):
                return slot_psum, blk, 0

            run_pass(
                metaL3, idx3_sb, vals3_sb, rb3_sb,
                lambda bank: tables[2][
                    bank * BANKROWS : bank * BANKROWS + min(BANKROWS, NPHYS - bank * BANKROWS), :
                ],
                psum_of_slot_blk, flags3, "l3",
            )
            run_pass(
                metaSyn, idxs_sb, valss_sb, rbs_sb,
                lambda bank: acc_dram[:, :],
                psum_of_slot_blk, flags3, "syn",
            )

            # drain slot PSUM -> ex_bounce -> AllGather
            slot_sb = finp.tile([128, NBLK3 * EMB], F32, tag="slot_sb")
            nc.scalar.copy(out=slot_sb[:, :], in_=slot_psum[:, :])
            nc.sync.dma_start(
                ex_bounce[:, :].rearrange("(p x) e -> p (x e)", p=128), slot_sb[:]
            )
            nc.gpsimd.collective_compute(
                "AllGather",
                mybir.AluOpType.bypass,
                ins=[ex_bounce[:, :].opt()],
                outs=[ex_full[:, :].opt()],
                replica_groups=[list(range(NC))],
            )

            # ================= final extraction + GEMM =================
            exu_sb = finp.tile([128, OUT_ROWS // 16], I16, tag="exu")
            exi_sb = finp.tile([128, BATCH // 16], I16, tag="exi")
            nc.sync.dma_start(exu_sb[:], exu_in[:])
            nc.sync.dma_start(exi_sb[:], exi_in[:])
            nc.gpsimd.tensor_copy(out=dummy16[:, :1], in_=exu_sb[:, :1])
            nc.gpsimd.tensor_copy(out=dummy16[:, :1], in_=exi_sb[:, :1])

            u_sb = finp.tile([128, (OUT_ROWS // 128) * EMB], F32, tag="u")
            i_sb = finp.tile([128, (BATCH // 128) * EMB], F32, tag="i")
            issue_gather(u_sb, 0, OUT_ROWS // BLK, ex_full[:, :], exu_sb[:, :])
            for part in range(2):
                issue_gather(
                    i_sb, part * 8 * EMB, 8, ex_full[:, :],
                    exi_sb[:, part * 64 : (part + 1) * 64],
                )
            ut = finp.tile([64, (OUT_ROWS // 128) * 128], BF16, tag="ut")
            it = finp.tile([64, (BATCH // 128) * 128], BF16, tag="it")
            for t in range(OUT_ROWS // 128):
                tp = fpsump.tile([64, 128], F32, tag="tp", name=f"tpu_{t}")
                nc.tensor.transpose(out=tp[:, :], in_=u_sb[:, t * EMB : (t + 1) * EMB], identity=ident_sb[:, :])
                nc.vector.tensor_copy(out=ut[:, t * 128 : (t + 1) * 128], in_=tp[:, :])
            for t in range(BATCH // 128):
                tp = fpsump.tile([64, 128], F32, tag="tp", name=f"tpi_{t}")
                nc.tensor.transpose(out=tp[:, :], in_=i_sb[:, t * EMB : (t + 1) * EMB], identity=ident_sb[:, :])
                nc.vector.tensor_copy(out=it[:, t * 128 : (t + 1) * 128], in_=tp[:, :])
            for t in range(OUT_ROWS // 128):
                for q in range(BATCH // 512):
                    po = fpsump.tile([128, 512], F32, tag="po", name=f"po_{t}_{q}")
                    nc.tensor.matmul(
                        out=po[:, :],
                        lhsT=ut[:, t * 128 : (t + 1) * 128],
                        rhs=it[:, q * 512 : (q + 1) * 512],
                        start=True, stop=True,
                    )
                    ob = finp.tile([128, 512], F32, tag="ob", name=f"ob_{t}_{q}")
                    nc.scalar.activation(
                        out=ob[:, :], in_=po[:, :],
                        func=mybir.ActivationFunctionType.Sigmoid,
                        scale=1.0 / ((N_LAYERS + 1) ** 2),
                    )
                    nc.sync.dma_start(
                        out_ext[t * 128 : (t + 1) * 128, q * 512 : (q + 1) * 512],
                        ob[:, :],
                    )
    nc.compile()
    return nc


LAST_EXEC_NS = None
LAST_RES = None


def _ensure_trace_hook():
    """Install the axon NTFF profile hook if the image's antenv lacks it.

    Mirrors trn_agent_boot.trn_boot's step 6 (which degrades silently when
    antenv.axon_hooks is missing). Best-effort: any failure leaves tracing
    disabled, which run_bass_kernel_spmd already tolerates.
    """
    try:
        from antenv.axon_hooks import get_axon_ntff_profile_hook  # noqa: F401

        return  # real module present; boot already handled it
    except ImportError:
        pass
    try:
        import contextlib
        import ctypes
        import types

        import antenv

        lib = ctypes.CDLL("/opt/axon/libaxon_pjrt.so")
        if not hasattr(lib, "axon_start_nrt_profile"):
            return
        lib.axon_start_nrt_profile.argtypes = [
            ctypes.POINTER(ctypes.c_int64),
            ctypes.c_size_t,
        ]
        lib.axon_start_nrt_profile.restype = ctypes.c_int64
        lib.axon_stop_nrt_profile.argtypes = [ctypes.c_char_p]
        lib.axon_stop_nrt_profile.restype = ctypes.c_int64

        @contextlib.contextmanager
        def _hook(output_dir, device_ids):
            import jax

            jax.devices()
            if device_ids:
                ids = (ctypes.c_int64 * len(device_ids))(*device_ids)
                rc = lib.axon_start_nrt_profile(ids, len(device_ids))
            else:
                rc = lib.axon_start_nrt_profile(None, 0)
            if rc != 0:
                raise RuntimeError(f"axon_start_nrt_profile rc={rc}")
            try:
                yield
            finally:
                n = lib.axon_stop_nrt_profile(str(output_dir).encode())
                if n <= 0:
                    print(f"profile: {n} ntff files in {output_dir}")

        mod = types.ModuleType("antenv.axon_hooks")
        mod._hook = _hook
        mod.get_axon_ntff_profile_hook = lambda: mod._hook
        mod.set_axon_ntff_profile_hook = lambda h: setattr(mod, "_hook", h)
        sys.modules["antenv.axon_hooks"] = mod
        antenv.axon_hooks = mod
    except Exception:
        pass


def kernel(user_emb, item_emb, adj_vals, adj_rows, adj_cols, users, items):
    global LAST_EXEC_NS, LAST_RES
    user_emb = np.asarray(user_emb, dtype=np.float32)
    item_emb = np.asarray(item_emb, dtype=np.float32)

    g = _prep_graph(adj_vals, adj_rows, adj_cols, users, items)

    all_emb = np.concatenate([user_emb, item_emb], axis=0)
    table0 = np.zeros((NPHYS, EMB), dtype=np.float32)
    table0[_phys(np.arange(N_NODES))] = all_emb

    iota = np.tile(np.arange(128, dtype=_BF16NP)[None, :], (128, 1))
    ident = np.eye(128, dtype=np.float32)

    nc = _build(g)

    colsL1 = _slot_cols(g["arrL1"], g["metaL1"]["S_total"])
    colsL2 = _slot_cols(g["arrL2"], g["metaL2"]["S_total"])
    colsL3 = _slot_cols(g["arrL3"], g["metaL3"]["S_total"])
    colsSyn = _slot_cols(g["arrSyn"], g["metaSyn"]["S_total"])

    in_maps = []
    for m in range(NC):
        i1, v1, r1 = colsL1[m]
        i2, v2, r2 = colsL2[m]
        i3, v3, r3 = colsL3[m]
        isn, vsn, rsn = colsSyn[m]
        in_maps.append(
            {
                "table0": table0,
                "acc0": table0[m * SHARD_PAD : (m + 1) * SHARD_PAD],
                "idx1": i1, "vals1": v1, "rb1": r1,
                "idx2": i2, "vals2": v2, "rb2": r2,
                "idx3": i3, "vals3": v3, "rb3": r3,
                "idxs": isn, "valss": vsn, "rbs": rsn,
                "iota": iota, "ident": ident,
                "exu": _wrap_idx(g["exu"][m]),
                "exi": _wrap_idx(g["exi"]),
            }
        )

    _ensure_trace_hook()
    try:
        res = run_bass_kernel_spmd(nc, in_maps, core_ids=list(range(NC)), trace=True)
        LAST_EXEC_NS = res.exec_time_ns
    except Exception:
        res = run_bass_kernel_spmd(nc, in_maps, core_ids=list(range(NC)))
        LAST_EXEC_NS = None
    LAST_RES = res
    out = np.concatenate([res.results[m]["out"] for m in range(NC)], axis=0)
    return out.astype(np.float32)

